# revision 113
# speedup vs baseline: 1.6135x; 1.0014x over previous
"""DiT block kernel for Trainium2 (Bass/Tile), 8-core data parallel.

Shapes (hardcoded from the problem spec):
  x: (8, 1024, 1152), t_emb: (8, 1152)
  w_qkv (1152, 3456), w_proj (1152, 1152), w_fc1 (1152, 4608),
  w_fc2 (4608, 1152), w_ada (1152, 6912) + biases.

Strategy: batch-parallel across 8 cores (one batch element each, no
collectives). Activations live transposed [D on partitions, tokens free].
The large matmuls (qkv, attention AV, proj, fc1, fc2) run in fp8e4 with
DoubleRow perf mode (two 128-row k-tiles contracted per instruction);
scale factors for fp8 range are folded into the existing activation
bias/scale stages so no extra elementwise work is added.  LayerNorm
statistics reduce over the partition axis via ones-vector f32r matmuls;
softmax runs transposed (keys on partitions) with denominators collected
through a ones-column appended to V and a fused divide.  q/k are produced
per-head directly (M=72 matmuls cost the same per column as M=128), so
attention needs no partition-crossing gather DMAs.  Weights stream
through big staged f32 DMA loads (few, large transfers) and are
converted on-chip; ada (error-sensitive) stays f32r.
"""

import threading
from contextlib import ExitStack

import numpy as np

import concourse.bass as bass
import concourse.mybir as mybir
import concourse.tile as tile
from concourse import bacc
from concourse.bass_utils import run_bass_kernel_spmd
from concourse.masks import make_identity

F32 = mybir.dt.float32
F32R = mybir.dt.float32r
BF16 = mybir.dt.bfloat16
FP8 = mybir.dt.float8e4
AF = mybir.ActivationFunctionType
ALU = mybir.AluOpType
DR = mybir.MatmulPerfMode.DoubleRow

NCORES = 8
D = 1152
NT = 1024
KT = D // 128       # 9
KTP = KT + 1        # padded to even for DoubleRow pairs
H = 16
HD = 72
HID = 4 * D
MH = HID // 128     # 36
EPS = 1e-6
ISC = 1.0 / float(np.sqrt(HD))

# fp8 scale factors
WS = 64.0           # weights
AS = 8.0            # modulated activations (mod1/mod2)
QS = 2.0            # q/k
PS = 4.0            # attention output
ES = ISC / (QS * QS)  # exp() input scale applied to the scores psum

# v output column slices aligned to head boundaries
V_SLICES = [(0, 432, 0, 6), (432, 864, 6, 12), (864, 1152, 12, 16)]


def _r(ap):
    return ap.bitcast(F32R)


def _build_program():
    nc = bacc.Bacc(
        "TRN2", target_bir_lowering=False, debug=False, enable_asserts=False
    )
    ins = {}
    ins["x"] = nc.dram_tensor("x", [NT, D], F32, kind="ExternalInput").ap()
    ins["t_emb"] = nc.dram_tensor("t_emb", [D], F32, kind="ExternalInput").ap()
    for name, shape in [
        ("w_qkv", [D, 3 * D]), ("b_qkv", [3 * D]),
        ("w_proj", [D, D]), ("b_proj", [D]),
        ("w_fc1", [D, HID]), ("b_fc1", [HID]),
        ("w_fc2", [HID, D]), ("b_fc2", [D]),
        ("w_ada", [D, 6 * D]), ("b_ada", [6 * D]),
    ]:
        ins[name] = nc.dram_tensor(name, shape, F32, kind="ExternalInput").ap()
    out_dram = nc.dram_tensor("out", [NT, D], F32, kind="ExternalOutput").ap()

    with tile.TileContext(nc) as tc:
        _body(tc, ins, out_dram)
    nc.compile()
    return nc


def _ln_stats(tc, nc, src, ones_col, pst, pln, ps_st, halves=(0, 1),
              st=None):
    """Return st[n] = [mean; rstd] rows [1, 2, 512] per 512-token half,
    reducing over the partition (D) axis of src [128, KT, NT] f32."""
    ps_x, ps_q = {}, {}
    if st is None:
        st = {}
    for n in halves:
        nsl = slice(n * 512, (n + 1) * 512)
        ps_x[n] = ps_st.tile([1, 512], F32, tag="st", name=f"psx{n}")
        ps_q[n] = ps_st.tile([1, 512], F32, tag="st", name=f"psq{n}")
        for k in range(KT):
            xb = pln.tile([128, 512], BF16, tag="xb", bufs=2, name="xb")
            nc.scalar.copy(xb[:, :], src[:, k, nsl])
            sq = pln.tile([128, 512], BF16, tag="sq", bufs=2, name="sq")
            nc.vector.tensor_mul(sq[:, :], src[:, k, nsl], src[:, k, nsl])
            nc.tensor.matmul(
                ps_x[n][:, :], ones_col[:, :], xb[:, :],
                start=(k == 0), stop=(k == KT - 1), skip_group_check=True,
            )
            nc.tensor.matmul(
                ps_q[n][:, :], ones_col[:, :], sq[:, :],
                start=(k == 0), stop=(k == KT - 1), skip_group_check=True,
            )
    eps_sb = pst.tile([1, 1], F32, tag="eps", bufs=1, name="eps_sb")
    nc.vector.memset(eps_sb[:, :], EPS)
    for n in halves:
        st[n] = pst.tile([1, 2, 512], F32, tag="lnst", bufs=2, name=f"st{n}")
        nc.vector.tensor_scalar_mul(st[n][:, 0, :], ps_x[n][:, :], 1.0 / D)
        work = pst.tile([1, 512], F32, tag="lnwork", bufs=2, name="work")
        nc.vector.tensor_mul(work[:, :], st[n][:, 0, :], st[n][:, 0, :])
        nc.vector.scalar_tensor_tensor(
            st[n][:, 1, :], ps_q[n][:, :], 1.0 / D, work[:, :],
            ALU.mult, ALU.subtract,
        )
        nc.scalar.activation(st[n][:, 1, :], st[n][:, 1, :], AF.Sqrt,
                             bias=eps_sb[:, :], scale=1.0)
        nc.vector.reciprocal(st[n][:, 1, :], st[n][:, 1, :])
    return st


def _ln_apply(tc, nc, src, dst, st, ada_pp, sh_c, sc_c, pln,
              halves=(0, 1)):
    """dst[:,k,nsl] (fp8) = ((src-mean)*rstd) * ada[sc_c] + ada[sh_c]
    (ada params pre-scaled by AS)."""
    for n in halves:
        nsl = slice(n * 512, (n + 1) * 512)
        meanB = pln.tile([128, 512], F32, tag="meanB", bufs=2, name="meanB")
        rstdB = pln.tile([128, 512], F32, tag="rstdB", bufs=2, name="rstdB")
        nc.gpsimd.partition_broadcast(meanB[:, :], st[n][:, 0, :])
        nc.gpsimd.partition_broadcast(rstdB[:, :], st[n][:, 1, :])
        for k in range(KT):
            t1 = pln.tile([128, 512], F32, tag="lnt1", bufs=3, name="t1")
            nc.vector.tensor_sub(t1[:, :], src[:, k, nsl], meanB[:, :])
            nc.vector.tensor_mul(t1[:, :], t1[:, :], rstdB[:, :])
            nc.gpsimd.tensor_scalar(
                dst[:, k, nsl], t1[:, :],
                ada_pp[:, sc_c, k:k + 1], ada_pp[:, sh_c, k:k + 1],
                ALU.mult, ALU.add,
            )


def _body(tc, ins, out_dram):
    nc = tc.nc
    ctx = ExitStack()
    with ctx:
        dram = ctx.enter_context(tc.tile_pool(name="dram", bufs=1, space="DRAM"))
        ada_dr = dram.tile([6 * D], F32)
        w1f8_dr = dram.tile([18, 128, KT, 256], FP8)

        pers = ctx.enter_context(tc.tile_pool(name="pers", bufs=1))
        ident = pers.tile([128, 128], F32)
        make_identity(nc, ident[:, :])
        ones_col = pers.tile([128, 1], BF16)
        nc.vector.memset(ones_col[:, :], 1.0)
        ones_row = pers.tile([1, 128], BF16)
        nc.vector.memset(ones_row[:, :], 1.0)

        t_pp = pers.tile([128, KT], F32)
        nc.sync.dma_start(t_pp[:, :], ins["t_emb"].rearrange("(k p) -> p k", p=128))
        t_pr = pers.tile([128, KT], F32R)
        nc.scalar.activation(t_pr[:, :], t_pp[:, :], AF.Silu)

        bq_s = pers.tile([72, H], F32)
        bk_s = pers.tile([72, H], F32)
        bv_row = pers.tile([1, D], F32)
        bv_b = pers.tile([1, D], BF16)
        bproj_pp = pers.tile([128, KT], F32)
        bfc1_pp = pers.tile([128, MH], F32)
        bfc2_pp = pers.tile([128, KT], F32)
        bada_pp = pers.tile([128, 6, KT], F32)
        ada_pp = pers.tile([128, 6, KT], F32)

        def emit_bias_loads():
            nc.sync.dma_start(
                bq_s[:, :], ins["b_qkv"][0:D].rearrange("(h p) -> p h", p=72))
            nc.sync.dma_start(
                bk_s[:, :], ins["b_qkv"][D:2 * D].rearrange("(h p) -> p h", p=72))
            nc.sync.dma_start(
                bv_row[:, :],
                ins["b_qkv"][2 * D:3 * D].rearrange("(a b) -> a b", a=1))
            # bv enters the v accumulation in (AS*WS)-scaled psum units
            nc.vector.tensor_scalar_mul(bv_b[:, :], bv_row[:, :], AS * WS)
            nc.sync.dma_start(
                bproj_pp[:, :], ins["b_proj"].rearrange("(m p) -> p m", p=128))
            nc.sync.dma_start(
                bfc1_pp[:, :], ins["b_fc1"].rearrange("(m p) -> p m", p=128))
            nc.sync.dma_start(
                bfc2_pp[:, :], ins["b_fc2"].rearrange("(m p) -> p m", p=128))
            nc.sync.dma_start(
                bada_pp[:, :, :],
                ins["b_ada"].rearrange("(c k p) -> p c k", k=KT, p=128))
            # pre-scale q/k biases by QS (folded into the psum->fp8 copies)
            nc.vector.tensor_scalar_mul(bq_s[:, :], bq_s[:, :], QS)
            nc.vector.tensor_scalar_mul(bk_s[:, :], bk_s[:, :], QS)

        xT = pers.tile([128, KT, NT], F32)      # becomes x2T after residual 1
        mod12T = pers.tile([128, KTP, NT], FP8)  # mod1T, later reused as mod2T
        nc.gpsimd.memset(mod12T[:, KT, :], 0.0)  # DoubleRow pad k-tile

        # ================= phase A: x load/transpose, ada, LN1 ==============

        def emit_ada_chunk(c, p1w, ps_pro, ps_bufs=2):
            """chunk c covers w_ada cols [c*384, (c+1)*384); param p=c//3."""
            wst = p1w.tile([128, KT, 384], F32R, tag="adast", bufs=2, name="wst")
            nc.sync.dma_start(
                wst[:, :, :],
                ins["w_ada"][:, c * 384:(c + 1) * 384]
                .rearrange("(k p) m -> p k m", p=128).bitcast(F32R),
            )
            pa = ps_pro.tile([1, 384], F32, tag="psada", bufs=ps_bufs,
                             name="pa")
            for k in range(KT):
                nc.tensor.matmul(
                    pa[:, :], t_pr[:, k:k + 1], wst[:, k, :],
                    start=(k == 0), stop=(k == KT - 1),
                )
            asb = p1w.tile([1, 384], F32, tag="asb", bufs=3, name="asb")
            nc.vector.tensor_copy(asb[:, :], pa[:, :])
            nc.scalar.dma_start(
                ada_dr[c * 384:(c + 1) * 384].rearrange("(a b) -> a b", a=1),
                asb[0:1, :],
            )

        def emit_ada_pp_load(cs):
            """Load+finalize ada params cs (list) into ada_pp; params 0/1
            (shift_a/scale_a) and 3/4 are pre-scaled by AS; 1/4 get +1."""
            for c in cs:
                nc.scalar.dma_start(
                    ada_pp[:, c, :],
                    ada_dr[c * D:(c + 1) * D].rearrange("(k p) -> p k", p=128),
                )
            lo, hi = min(cs), max(cs) + 1
            nc.vector.tensor_add(ada_pp[:, lo:hi, :], ada_pp[:, lo:hi, :],
                                 bada_pp[:, lo:hi, :])
            for c in cs:
                if c in (1, 4):
                    nc.vector.tensor_scalar_add(ada_pp[:, c, :],
                                                ada_pp[:, c, :], 1.0)
                if c in (0, 1, 3, 4):
                    nc.vector.tensor_scalar_mul(ada_pp[:, c, :],
                                                ada_pp[:, c, :], AS)

        with tc.tile_pool(name="p1w", bufs=1) as p1w, \
             tc.tile_pool(name="pxin", bufs=3) as pxin, \
             tc.tile_pool(name="ps_pro", bufs=2, space="PSUM") as ps_pro, \
             tc.tile_pool(name="ps_tr", bufs=2, space="PSUM") as ps_tr:

            def emit_transpose_block(tt):
                xin = pxin.tile([128, D], F32, tag="xin", name="xin")
                nc.sync.dma_start(
                    xin[:, :], ins["x"][tt * 128:(tt + 1) * 128, :])
                for kd in range(KT):
                    pt = ps_tr.tile([128, 128], F32, tag="ptr", name="pt")
                    nc.tensor.transpose(
                        pt[:, :], xin[:, kd * 128:(kd + 1) * 128], ident[:, :])
                    tsl = slice(tt * 128, (tt + 1) * 128)
                    if kd % 2 == 0:
                        nc.vector.tensor_copy(xT[:, kd, tsl], pt[:, :])
                    else:
                        nc.scalar.copy(xT[:, kd, tsl], pt[:, :])

            for i in range(8):
                emit_transpose_block(i)
                if i == 0:
                    emit_bias_loads()
                if i < 6:
                    emit_ada_chunk(i, p1w, ps_pro)
            emit_ada_pp_load([0, 1])

        # ====== phase B part 1: qkv weight loads + converts (emitted before
        # LN1 so SP streams the loads while ada finishes / LN runs) =========
        es_qk = ExitStack()
        pqk8 = es_qk.enter_context(tc.tile_pool(name="pqk8", bufs=1))
        wq8 = pqk8.tile([128, KTP, D], FP8, name="wq8")
        wk8 = pqk8.tile([128, KTP, D], FP8, name="wk8")
        nc.gpsimd.memset(wq8[:, KT, :], 0.0)
        nc.gpsimd.memset(wk8[:, KT, :], 0.0)

        es_att = ExitStack()
        patt = es_att.enter_context(tc.tile_pool(name="patt", bufs=1, side="right"))
        attn_hs = patt.tile([72, H, NT], FP8, name="attn_hs")
        es_wp = ExitStack()
        pwp8 = es_wp.enter_context(
            tc.tile_pool(name="pwp8", bufs=1, side="right"))
        wp8 = pwp8.tile([72, H, D], FP8, name="wp8")
        es_va = ExitStack()
        pva = es_va.enter_context(tc.tile_pool(name="pva", bufs=1, side="right"))
        v_aug = pva.tile([128, NT // 128, H, 97], FP8, name="v_aug")
        nc.gpsimd.memset(v_aug[:, :, :, HD:97], 0.0)
        nc.gpsimd.memset(v_aug[:, :, :, 96:97], 1.0)

        es_b = ExitStack()
        pwst = es_b.enter_context(tc.tile_pool(name="pwst", bufs=1))
        wv8 = pwst.tile([128, KTP, D], FP8, tag="wv8", bufs=1, name="wv8")
        nc.gpsimd.memset(wv8[:, KT, :], 0.0)
        engs = ["act", "dve", "act", "dve", "act", "dve"]
        for j, (dst8, c0) in enumerate(((wq8, 0), (wk8, D), (wv8, 2 * D))):
            for half in range(2):
                msl = slice(half * 576, (half + 1) * 576)
                wst = pwst.tile([128, KT, 576], F32, tag="wst", bufs=2,
                                name="wst")
                nc.sync.dma_start(
                    wst[:, :, :],
                    ins["w_qkv"][:, c0 + half * 576:c0 + (half + 1) * 576]
                    .rearrange("(k p) m -> p k m", p=128),
                )
                eng = engs[j * 2 + half]
                for kk in range(3):
                    ksl = slice(kk * 3, kk * 3 + 3)
                    if eng == "act":
                        nc.scalar.activation(
                            dst8[:, ksl, msl], wst[:, ksl, :],
                            AF.Identity, scale=WS)
                    elif eng == "dve":
                        nc.vector.tensor_scalar_mul(
                            dst8[:, ksl, msl], wst[:, ksl, :], WS)
                    else:
                        nc.gpsimd.tensor_scalar_mul(
                            dst8[:, ksl, msl], wst[:, ksl, :], WS)

        # ====== LN1 (per-half, interleaved with v matmuls) ==================
        with tc.tile_pool(name="pst", bufs=1) as pst, \
             tc.tile_pool(name="pln", bufs=1) as pln, \
             tc.tile_pool(name="ps_st", bufs=4, space="PSUM") as ps_st, \
             tc.tile_pool(name="ps_v", bufs=3, space="PSUM") as ps_v:

            def v_block(tts):
                for tt in tts:
                    tsl = slice(tt * 128, (tt + 1) * 128)
                    for si, (c0, c1, h0, h1) in enumerate(V_SLICES):
                        pmv = ps_v.tile([128, 512], F32, tag="mv", name="pmv")
                        for i in range(KTP // 2):
                            nc.tensor.matmul(
                                pmv[:, 0:c1 - c0],
                                mod12T[:, 2 * i:2 * i + 2, tsl],
                                wv8[:, 2 * i:2 * i + 2, c0:c1],
                                start=(i == 0), stop=False, perf_mode=DR,
                                skip_group_check=True,
                            )
                        nc.tensor.matmul(
                            pmv[:, 0:c1 - c0], ones_row[:, :],
                            bv_b[:, c0:c1],
                            start=False, stop=True, skip_group_check=True,
                        )
                        vsrc = pmv[:, 0:c1 - c0].rearrange(
                            "p (h d) -> p h d", d=HD)
                        nc.vector.tensor_scalar_mul(
                            v_aug[:, tt, h0:h1, 0:HD], vsrc, 1.0 / (AS * WS))

            st1 = {}
            _ln_stats(tc, nc, xT, ones_col, pst, pln, ps_st, halves=(0,),
                      st=st1)
            _ln_apply(tc, nc, xT, mod12T, st1, ada_pp, 0, 1, pln, halves=(0,))
            _ln_stats(tc, nc, xT, ones_col, pst, pln, ps_st, halves=(1,),
                      st=st1)
            v_block(range(0, 4))
            _ln_apply(tc, nc, xT, mod12T, st1, ada_pp, 0, 1, pln, halves=(1,))
            v_block(range(4, 8))
        es_b.close()

        # ================= phase C: attention ===============================
        with tc.tile_pool(name="p3w", bufs=1) as p3w, \
             tc.tile_pool(name="pexp", bufs=1) as pexp, \
             tc.tile_pool(name="pat3", bufs=1) as pat3, \
             tc.tile_pool(name="ps_qk", bufs=2, space="PSUM") as ps_qk, \
             tc.tile_pool(name="ps_s", bufs=2, space="PSUM") as ps_s, \
             tc.tile_pool(name="ps_av", bufs=1, space="PSUM") as ps_av, \
             tc.tile_pool(name="ps_pa", bufs=1, space="PSUM") as ps_pa:

            def emit_fc1_stream(j):
                f1st = p3w.tile([128, KT, 256], F32, tag="f1st",
                                bufs=2, name="f1st")
                nc.sync.dma_start(
                    f1st[:, :, :],
                    ins["w_fc1"][:, j * 256:(j + 1) * 256]
                    .rearrange("(k p) m -> p k m", p=128),
                )
                f18o = p3w.tile([128, KT, 256], FP8, tag="f18o",
                                bufs=2, name="f18o")
                nc.gpsimd.tensor_scalar_mul(
                    f18o[:, :, :], f1st[:, :, :], WS)
                nc.scalar.dma_start(w1f8_dr[j, :, :, :], f18o[:, :, :])

            def emit_wp_stream(c):
                # reuse the f1st staging tag: [128, KT*256] bytes == 16*144
                wpt = p3w.tile([128, KT, 256], F32, tag="f1st", bufs=2,
                               name="wpt")
                wpv = (wpt[:, :, :].rearrange("p k m -> p (k m)")[0:72, :]
                       .rearrange("p (h m) -> p h m", h=H))
                msl = slice(c * 144, (c + 1) * 144)
                nc.sync.dma_start(
                    wpv[:, :, :],
                    ins["w_proj"][:, msl].rearrange("(h p) m -> p h m", p=72),
                )
                nc.vector.tensor_scalar_mul(
                    wp8[:, :, msl], wpv[:, :, :], WS)

            def emit_wp_stream(c):
                # reuse the f1st staging tag: KT*256 f32 bytes == 16*144
                wpt = p3w.tile([128, KT, 256], F32, tag="f1st", bufs=2,
                               name="wpt")
                wpv = (wpt[:, :, :].rearrange("p k m -> p (k m)")[0:72, :]
                       .rearrange("p (h m) -> p h m", h=H))
                msl = slice(c * 144, (c + 1) * 144)
                nc.sync.dma_start(
                    wpv[:, :, :],
                    ins["w_proj"][:, msl].rearrange("(h p) m -> p h m", p=72),
                )
                nc.vector.tensor_scalar_mul(
                    wp8[:, :, msl], wpv[:, :, :], WS)

            def emit_filler(h):
                # late ada chunks; fc1 fp8 stream-convert to DRAM
                if h % 4 != 3:
                    emit_ada_chunk(6 + h - h // 4, p3w, ps_pa, ps_bufs=1)
                if h == 15:
                    emit_ada_pp_load([2, 3])
                    emit_ada_pp_load([4, 5])
                if 2 <= h:
                    js = ([2 * h - 4, 2 * h - 3] if h < 6
                          else [h + 2])
                    for j in js:
                        emit_fc1_stream(j)
                if h >= 12:
                    emit_wp_stream(h - 12)

            for h in range(H):
                emit_filler(h)
                q_h = pat3.tile([72, NT], FP8, tag="qh", bufs=2, name="q_h")
                k_h = pat3.tile([72, NT], FP8, tag="kh", bufs=2, name="k_h")
                for n in range(2):
                    nsl = slice(n * 512, (n + 1) * 512)
                    pq = ps_qk.tile([72, 512], F32, tag="qk", name="pq")
                    for i in range(KTP // 2):
                        nc.tensor.matmul(
                            pq[:, :],
                            wq8[:, 2 * i:2 * i + 2, h * HD:(h + 1) * HD],
                            mod12T[:, 2 * i:2 * i + 2, nsl],
                            start=(i == 0), stop=(i == KTP // 2 - 1),
                            perf_mode=DR,
                        )
                    nc.vector.tensor_scalar(
                        q_h[:, nsl], pq[:, :], QS / (AS * WS),
                        bq_s[:, h:h + 1], ALU.mult, ALU.add,
                    )
                for n in range(2):
                    nsl = slice(n * 512, (n + 1) * 512)
                    pk = ps_qk.tile([72, 512], F32, tag="qk", name="pk")
                    for i in range(KTP // 2):
                        nc.tensor.matmul(
                            pk[:, :],
                            wk8[:, 2 * i:2 * i + 2, h * HD:(h + 1) * HD],
                            mod12T[:, 2 * i:2 * i + 2, nsl],
                            start=(i == 0), stop=(i == KTP // 2 - 1),
                            perf_mode=DR,
                        )
                    nc.vector.tensor_scalar(
                        k_h[:, nsl], pk[:, :], QS / (AS * WS),
                        bk_s[:, h:h + 1], ALU.mult, ALU.add,
                    )
                for n in range(2):
                    nsl = slice(n * 512, (n + 1) * 512)
                    exp_hn = pexp.tile([128, NT // 128, 512], FP8, tag="exp",
                                       bufs=3, name="exp_hn")
                    for kp in range(NT // 256):
                        pss = ps_s.tile([128, 2, 512], F32, tag="s",
                                        name="pss")
                        for j in range(2):
                            kt_i = 2 * kp + j
                            nc.tensor.matmul(
                                pss[:, j, :],
                                k_h[:, kt_i * 128:(kt_i + 1) * 128],
                                q_h[:, nsl], start=True, stop=True,
                            )
                        nc.scalar.activation(
                            exp_hn[:, 2 * kp:2 * kp + 2, :],
                            pss[:, :, :], AF.Exp, scale=ES)
                    pav = ps_av.tile([97, 512], F32, tag="av", name="pav")
                    for i in range(NT // 256):
                        nc.tensor.matmul(
                            pav[:, :],
                            v_aug[:, 2 * i:2 * i + 2, h, :],
                            exp_hn[:, 2 * i:2 * i + 2, :],
                            start=(i == 0), stop=(i == NT // 256 - 1),
                            perf_mode=DR,
                        )
                    den = pat3.tile([1, 512], F32, tag="den", bufs=3,
                                    name="den")
                    nc.vector.tensor_scalar_mul(den[:, :], pav[96:97, :],
                                                1.0 / PS)
                    nc.vector.reciprocal(den[:, :], den[:, :])
                    denB = pat3.tile([72, 512], F32, tag="denB", bufs=3,
                                     name="denB")
                    nc.gpsimd.partition_broadcast(denB[:, :], den[:, :])
                    nc.vector.tensor_mul(
                        attn_hs[:, h, nsl], pav[0:HD, :], denB[:, :])
        es_qk.close()  # wq8/wk8 no longer needed
        es_va.close()

        # ================= phase D: proj + residual + LN2 ===================
        es_w2 = ExitStack()
        pw2 = es_w2.enter_context(
            tc.tile_pool(name="pw2", bufs=1, side="right"))
        w2f8 = pw2.tile([128, KT, MH, 128], FP8, name="w2f8")

        with tc.tile_pool(name="p4", bufs=1) as p4, \
             tc.tile_pool(name="pst4", bufs=1) as pst4, \
             tc.tile_pool(name="pln4", bufs=1) as pln4:

            for i in range(4, 8):
                msl = slice(i * 144, (i + 1) * 144)
                wpst = p4.tile([72, H, 144], F32, tag="wpst", bufs=2,
                               name="wpst")
                nc.sync.dma_start(
                    wpst[:, :, :],
                    ins["w_proj"][:, msl].rearrange("(h p) m -> p h m", p=72),
                )
                for kk in range(2):
                    hsl = slice(kk * 8, kk * 8 + 8)
                    nc.vector.tensor_scalar_mul(
                        wp8[:, hsl, msl], wpst[:, hsl, :], WS)

            def emit_fc2_chunk(ch, eng, pool):
                f2s = pool.tile([128, MH, 64], F32, tag="f2s", bufs=2,
                                name="f2s")
                nc.sync.dma_start(
                    f2s[:, :, :],
                    ins["w_fc2"][:, ch * 64:(ch + 1) * 64]
                    .rearrange("(k p) m -> p k m", p=128),
                )
                eng.tensor_scalar_mul(
                    w2f8[:, ch // 2, :, (ch % 2) * 64:(ch % 2 + 1) * 64],
                    f2s[:, :, :], WS)

            st2 = {}
            with tc.tile_pool(name="ps_mm2", bufs=4, space="PSUM") as ps_mm2, \
                 tc.tile_pool(name="ps_st2", bufs=4, space="PSUM") as ps_st2:
                for n in range(2):
                    nsl = slice(n * 512, (n + 1) * 512)
                    for mo in range(KT):
                        if mo < 4:
                            ch = n * 4 + mo
                            eng = nc.gpsimd if ch % 2 else nc.vector
                            emit_fc2_chunk(ch, eng, p4)
                        pm2 = ps_mm2.tile([128, 512], F32, tag="mm2",
                                          name="pm2")
                        for i in range(H // 2):
                            nc.tensor.matmul(
                                pm2[:, :],
                                wp8[:, 2 * i:2 * i + 2,
                                    mo * 128:(mo + 1) * 128],
                                attn_hs[:, 2 * i:2 * i + 2, nsl],
                                start=(i == 0), stop=(i == H // 2 - 1),
                                perf_mode=DR,
                            )
                        t_sb = p4.tile([128, 512], F32, tag="tsb", bufs=3,
                                       name="t_sb")
                        nc.scalar.activation(
                            t_sb[:, :], pm2[:, :], AF.Identity,
                            bias=bproj_pp[:, mo:mo + 1], scale=1.0 / (PS * WS),
                        )
                        nc.vector.scalar_tensor_tensor(
                            xT[:, mo, nsl], t_sb[:, :],
                            ada_pp[:, 2, mo:mo + 1], xT[:, mo, nsl],
                            ALU.mult, ALU.add,
                        )
                    _ln_stats(tc, nc, xT, ones_col, pst4, pln4, ps_st2,
                              halves=(n,), st=st2)
                    _ln_apply(tc, nc, xT, mod12T, st2, ada_pp, 3, 4, pln4,
                              halves=(n,))

        # ================= phase E: FFN =====================================
        es_e = ExitStack()
        ph = es_e.enter_context(tc.tile_pool(name="ph", bufs=1))
        hT = ph.tile([128, MH, NT], FP8, name="hT")
        po = es_e.enter_context(tc.tile_pool(name="po", bufs=1))

        with tc.tile_pool(name="ps_f1", bufs=3, space="PSUM") as ps_f1, \
             tc.tile_pool(name="ps_f2", bufs=3, space="PSUM") as ps_f2, \
             tc.tile_pool(name="ps_tro", bufs=2, space="PSUM") as ps_tro:
            # fc1 in 18 chunks of 256 columns (2 m-tiles each), weights
            # already converted to fp8 in DRAM during the attention window
            with tc.tile_pool(name="p5a", bufs=1) as p5a:
                for ch in range(18):
                    f18 = p5a.tile([128, KTP, 256], FP8, tag="f18", bufs=3,
                                   name="f18")
                    nc.sync.dma_start(f18[:, 0:KT, :], w1f8_dr[ch, :, :, :])
                    nc.gpsimd.memset(f18[:, KT, :], 0.0)
                    if 2 <= ch < 12:
                        emit_fc2_chunk(6 + ch,
                                       nc.gpsimd if ch % 2 else nc.vector,
                                       p5a)
                    for m in range(2):
                        mo = ch * 2 + m
                        for n in range(2):
                            nsl = slice(n * 512, (n + 1) * 512)
                            pf1 = ps_f1.tile([128, 512], F32, tag="f1",
                                             name="pf1")
                            for i in range(KTP // 2):
                                nc.tensor.matmul(
                                    pf1[:, :],
                                    f18[:, 2 * i:2 * i + 2,
                                        m * 128:(m + 1) * 128],
                                    mod12T[:, 2 * i:2 * i + 2, nsl],
                                    start=(i == 0), stop=(i == KTP // 2 - 1),
                                    perf_mode=DR,
                                )
                            nc.scalar.activation(
                                hT[:, mo, nsl], pf1[:, :], AF.Gelu_apprx_tanh,
                                bias=bfc1_pp[:, mo:mo + 1],
                                scale=1.0 / (AS * WS),
                            )
            # fc2: weights already fp8-resident in SBUF (w2f8)
            with tc.tile_pool(name="p5b", bufs=1) as p5b:
                for mo in range(KT):
                    for n in range(2):
                        nsl = slice(n * 512, (n + 1) * 512)
                        pf2 = ps_f2.tile([128, 512], F32, tag="f2", name="pf2")
                        for i in range(MH // 2):
                            nc.tensor.matmul(
                                pf2[:, :], w2f8[:, mo, 2 * i:2 * i + 2, :],
                                hT[:, 2 * i:2 * i + 2, nsl],
                                start=(i == 0), stop=(i == MH // 2 - 1),
                                perf_mode=DR,
                            )
                        t2 = p5b.tile([128, 512], F32, tag="t2", bufs=4,
                                      name="t2")
                        nc.scalar.activation(
                            t2[:, :], pf2[:, :], AF.Identity,
                            bias=bfc2_pp[:, mo:mo + 1], scale=1.0 / WS,
                        )
                        nc.vector.scalar_tensor_tensor(
                            xT[:, mo, nsl], t2[:, :], ada_pp[:, 5, mo:mo + 1],
                            xT[:, mo, nsl], ALU.mult, ALU.add,
                        )
                    o_slab = po.tile([128, NT // 128, 128], F32, tag="osl",
                                     bufs=3, name="o_slab")
                    for tt in range(NT // 128):
                        pt = ps_tro.tile([128, 128], F32, tag="tro",
                                         name="pt6")
                        nc.tensor.transpose(
                            pt[:, :], xT[:, mo, tt * 128:(tt + 1) * 128],
                            ident[:, :],
                        )
                        dst = o_slab[:, tt, :]
                        nc.vector.tensor_copy(dst, pt[:, :])
                    nc.scalar.dma_start(
                        out_dram[:, mo * 128:(mo + 1) * 128]
                        .rearrange("(t p) m -> p t m", p=128),
                        o_slab[:, :, :])
        es_w2.close()
        es_wp.close()
        es_att.close()
        es_e.close()


_LOCK = threading.Lock()
_PROG = None


def _get_program():
    global _PROG
    with _LOCK:
        if _PROG is None:
            _PROG = _build_program()
    return _PROG


def _make_in_maps(inputs):
    arrs = {k: np.ascontiguousarray(np.asarray(v, dtype=np.float32))
            for k, v in inputs.items()}
    in_maps = []
    for c in range(NCORES):
        m = {k: v for k, v in arrs.items() if k not in ("x", "t_emb")}
        m["x"] = np.ascontiguousarray(arrs["x"][c])
        m["t_emb"] = np.ascontiguousarray(arrs["t_emb"][c])
        in_maps.append(m)
    return in_maps


def kernel(**inputs):
    nc = _get_program()
    res = run_bass_kernel_spmd(nc, _make_in_maps(inputs),
                               core_ids=list(range(NCORES)))
    return np.stack([r["out"] for r in res.results], axis=0)


def kernel_traced(inputs, **kw):
    """test-harness helper: returns full BassKernelResults with trace."""
    nc = _get_program()
    return run_bass_kernel_spmd(
        nc, _make_in_maps(inputs), core_ids=list(range(NCORES)), trace=True,
        **kw
    )


# revision 118
# speedup vs baseline: 1.6200x; 1.0040x over previous
"""DiT block kernel for Trainium2 (Bass/Tile), 8-core data parallel.

Shapes (hardcoded from the problem spec):
  x: (8, 1024, 1152), t_emb: (8, 1152)
  w_qkv (1152, 3456), w_proj (1152, 1152), w_fc1 (1152, 4608),
  w_fc2 (4608, 1152), w_ada (1152, 6912) + biases.

Strategy: batch-parallel across 8 cores (one batch element each, no
collectives). Activations live transposed [D on partitions, tokens free].
The large matmuls (qkv, attention AV, proj, fc1, fc2) run in fp8e4 with
DoubleRow perf mode (two 128-row k-tiles contracted per instruction);
scale factors for fp8 range are folded into the existing activation
bias/scale stages so no extra elementwise work is added.  LayerNorm
statistics reduce over the partition axis via ones-vector f32r matmuls;
softmax runs transposed (keys on partitions) with denominators collected
through a ones-column appended to V and a fused divide.  q/k are produced
per-head directly (M=72 matmuls cost the same per column as M=128), so
attention needs no partition-crossing gather DMAs.  Weights stream
through big staged f32 DMA loads (few, large transfers) and are
converted on-chip; ada (error-sensitive) stays f32r.
"""

import threading
from contextlib import ExitStack

import numpy as np

import concourse.bass as bass
import concourse.mybir as mybir
import concourse.tile as tile
from concourse import bacc
from concourse.bass_utils import run_bass_kernel_spmd
from concourse.masks import make_identity

F32 = mybir.dt.float32
F32R = mybir.dt.float32r
BF16 = mybir.dt.bfloat16
FP8 = mybir.dt.float8e4
AF = mybir.ActivationFunctionType
ALU = mybir.AluOpType
DR = mybir.MatmulPerfMode.DoubleRow

NCORES = 8
D = 1152
NT = 1024
KT = D // 128       # 9
KTP = KT + 1        # padded to even for DoubleRow pairs
H = 16
HD = 72
HID = 4 * D
MH = HID // 128     # 36
EPS = 1e-6
ISC = 1.0 / float(np.sqrt(HD))

# fp8 scale factors
WS = 64.0           # weights
AS = 8.0            # modulated activations (mod1/mod2)
QS = 2.0            # q/k
PS = 4.0            # attention output
ES = ISC / (QS * QS)  # exp() input scale applied to the scores psum

# v output column slices aligned to head boundaries
V_SLICES = [(0, 432, 0, 6), (432, 864, 6, 12), (864, 1152, 12, 16)]


def _r(ap):
    return ap.bitcast(F32R)


def _build_program():
    nc = bacc.Bacc(
        "TRN2", target_bir_lowering=False, debug=False, enable_asserts=False
    )
    ins = {}
    ins["x"] = nc.dram_tensor("x", [NT, D], F32, kind="ExternalInput").ap()
    ins["t_emb"] = nc.dram_tensor("t_emb", [D], F32, kind="ExternalInput").ap()
    for name, shape in [
        ("w_qkv", [D, 3 * D]), ("b_qkv", [3 * D]),
        ("w_proj", [D, D]), ("b_proj", [D]),
        ("w_fc1", [D, HID]), ("b_fc1", [HID]),
        ("w_fc2", [HID, D]), ("b_fc2", [D]),
        ("w_ada", [D, 6 * D]), ("b_ada", [6 * D]),
    ]:
        ins[name] = nc.dram_tensor(name, shape, F32, kind="ExternalInput").ap()
    out_dram = nc.dram_tensor("out", [NT, D], F32, kind="ExternalOutput").ap()

    with tile.TileContext(nc) as tc:
        _body(tc, ins, out_dram)
    nc.compile()
    return nc


def _ln_stats(tc, nc, src, ones_col, pst, pln, ps_st, halves=(0, 1),
              st=None):
    """Return st[n] = [mean; rstd] rows [1, 2, 512] per 512-token half,
    reducing over the partition (D) axis of src [128, KT, NT] f32."""
    ps_x, ps_q = {}, {}
    if st is None:
        st = {}
    for n in halves:
        nsl = slice(n * 512, (n + 1) * 512)
        ps_x[n] = ps_st.tile([1, 512], F32, tag="st", name=f"psx{n}")
        ps_q[n] = ps_st.tile([1, 512], F32, tag="st", name=f"psq{n}")
        for k in range(KT):
            xb = pln.tile([128, 512], BF16, tag="xb", bufs=2, name="xb")
            nc.scalar.copy(xb[:, :], src[:, k, nsl])
            sq = pln.tile([128, 512], BF16, tag="sq", bufs=2, name="sq")
            nc.vector.tensor_mul(sq[:, :], src[:, k, nsl], src[:, k, nsl])
            nc.tensor.matmul(
                ps_x[n][:, :], ones_col[:, :], xb[:, :],
                start=(k == 0), stop=(k == KT - 1), skip_group_check=True,
            )
            nc.tensor.matmul(
                ps_q[n][:, :], ones_col[:, :], sq[:, :],
                start=(k == 0), stop=(k == KT - 1), skip_group_check=True,
            )
    eps_sb = pst.tile([1, 1], F32, tag="eps", bufs=1, name="eps_sb")
    nc.vector.memset(eps_sb[:, :], EPS)
    for n in halves:
        st[n] = pst.tile([1, 2, 512], F32, tag="lnst", bufs=2, name=f"st{n}")
        nc.vector.tensor_scalar_mul(st[n][:, 0, :], ps_x[n][:, :], 1.0 / D)
        work = pst.tile([1, 512], F32, tag="lnwork", bufs=2, name="work")
        nc.vector.tensor_mul(work[:, :], st[n][:, 0, :], st[n][:, 0, :])
        nc.vector.scalar_tensor_tensor(
            st[n][:, 1, :], ps_q[n][:, :], 1.0 / D, work[:, :],
            ALU.mult, ALU.subtract,
        )
        nc.scalar.activation(st[n][:, 1, :], st[n][:, 1, :], AF.Sqrt,
                             bias=eps_sb[:, :], scale=1.0)
        nc.vector.reciprocal(st[n][:, 1, :], st[n][:, 1, :])
    return st


def _ln_apply(tc, nc, src, dst, st, ada_pp, sh_c, sc_c, pln,
              halves=(0, 1)):
    """dst[:,k,nsl] (fp8) = ((src-mean)*rstd) * ada[sc_c] + ada[sh_c]
    (ada params pre-scaled by AS)."""
    for n in halves:
        nsl = slice(n * 512, (n + 1) * 512)
        meanB = pln.tile([128, 512], F32, tag="meanB", bufs=2, name="meanB")
        rstdB = pln.tile([128, 512], F32, tag="rstdB", bufs=2, name="rstdB")
        nc.gpsimd.partition_broadcast(meanB[:, :], st[n][:, 0, :])
        nc.gpsimd.partition_broadcast(rstdB[:, :], st[n][:, 1, :])
        for k in range(KT):
            t1 = pln.tile([128, 512], F32, tag="lnt1", bufs=3, name="t1")
            nc.vector.tensor_sub(t1[:, :], src[:, k, nsl], meanB[:, :])
            nc.vector.tensor_mul(t1[:, :], t1[:, :], rstdB[:, :])
            nc.gpsimd.tensor_scalar(
                dst[:, k, nsl], t1[:, :],
                ada_pp[:, sc_c, k:k + 1], ada_pp[:, sh_c, k:k + 1],
                ALU.mult, ALU.add,
            )


def _body(tc, ins, out_dram):
    nc = tc.nc
    ctx = ExitStack()
    with ctx:
        dram = ctx.enter_context(tc.tile_pool(name="dram", bufs=1, space="DRAM"))
        ada_dr = dram.tile([6 * D], F32)
        w1f8_dr = dram.tile([18, 128, KT, 256], FP8)

        pers = ctx.enter_context(tc.tile_pool(name="pers", bufs=1))
        ident = pers.tile([128, 128], F32)
        make_identity(nc, ident[:, :])
        ones_col = pers.tile([128, 1], BF16)
        nc.vector.memset(ones_col[:, :], 1.0)
        ones_row = pers.tile([1, 128], BF16)
        nc.vector.memset(ones_row[:, :], 1.0)

        t_pp = pers.tile([128, KT], F32)
        nc.sync.dma_start(t_pp[:, :], ins["t_emb"].rearrange("(k p) -> p k", p=128))
        t_pr = pers.tile([128, KT], F32R)
        nc.scalar.activation(t_pr[:, :], t_pp[:, :], AF.Silu)

        bq_s = pers.tile([72, H], F32)
        bk_s = pers.tile([72, H], F32)
        bv_row = pers.tile([1, D], F32)
        bv_b = pers.tile([1, D], BF16)
        bproj_pp = pers.tile([128, KT], F32)
        bfc1_pp = pers.tile([128, MH], F32)
        bfc2_pp = pers.tile([128, KT], F32)
        bada_pp = pers.tile([128, 6, KT], F32)
        ada_pp = pers.tile([128, 6, KT], F32)

        def emit_bias_loads():
            nc.sync.dma_start(
                bq_s[:, :], ins["b_qkv"][0:D].rearrange("(h p) -> p h", p=72))
            nc.sync.dma_start(
                bk_s[:, :], ins["b_qkv"][D:2 * D].rearrange("(h p) -> p h", p=72))
            nc.sync.dma_start(
                bv_row[:, :],
                ins["b_qkv"][2 * D:3 * D].rearrange("(a b) -> a b", a=1))
            # bv enters the v accumulation in (AS*WS)-scaled psum units
            nc.vector.tensor_scalar_mul(bv_b[:, :], bv_row[:, :], AS * WS)
            nc.sync.dma_start(
                bproj_pp[:, :], ins["b_proj"].rearrange("(m p) -> p m", p=128))
            nc.sync.dma_start(
                bfc1_pp[:, :], ins["b_fc1"].rearrange("(m p) -> p m", p=128))
            nc.sync.dma_start(
                bfc2_pp[:, :], ins["b_fc2"].rearrange("(m p) -> p m", p=128))
            nc.sync.dma_start(
                bada_pp[:, :, :],
                ins["b_ada"].rearrange("(c k p) -> p c k", k=KT, p=128))
            # pre-scale q/k biases by QS (folded into the psum->fp8 copies)
            nc.vector.tensor_scalar_mul(bq_s[:, :], bq_s[:, :], QS)
            nc.vector.tensor_scalar_mul(bk_s[:, :], bk_s[:, :], QS)

        xT = pers.tile([128, KT, NT], F32)      # becomes x2T after residual 1
        mod12T = pers.tile([128, KTP, NT], FP8)  # mod1T, later reused as mod2T
        nc.gpsimd.memset(mod12T[:, KT, :], 0.0)  # DoubleRow pad k-tile

        # ================= phase A: x load/transpose, ada, LN1 ==============

        def emit_ada_chunk(c, p1w, ps_pro, ps_bufs=2):
            """chunk c covers w_ada cols [c*384, (c+1)*384); param p=c//3."""
            wst = p1w.tile([128, KT, 384], F32R, tag="adast", bufs=2, name="wst")
            nc.sync.dma_start(
                wst[:, :, :],
                ins["w_ada"][:, c * 384:(c + 1) * 384]
                .rearrange("(k p) m -> p k m", p=128).bitcast(F32R),
            )
            pa = ps_pro.tile([1, 384], F32, tag="psada", bufs=ps_bufs,
                             name="pa")
            for k in range(KT):
                nc.tensor.matmul(
                    pa[:, :], t_pr[:, k:k + 1], wst[:, k, :],
                    start=(k == 0), stop=(k == KT - 1),
                )
            asb = p1w.tile([1, 384], F32, tag="asb", bufs=3, name="asb")
            nc.vector.tensor_copy(asb[:, :], pa[:, :])
            nc.scalar.dma_start(
                ada_dr[c * 384:(c + 1) * 384].rearrange("(a b) -> a b", a=1),
                asb[0:1, :],
            )

        def emit_ada_pp_load(cs):
            """Load+finalize ada params cs (list) into ada_pp; params 0/1
            (shift_a/scale_a) and 3/4 are pre-scaled by AS; 1/4 get +1."""
            for c in cs:
                nc.scalar.dma_start(
                    ada_pp[:, c, :],
                    ada_dr[c * D:(c + 1) * D].rearrange("(k p) -> p k", p=128),
                )
            lo, hi = min(cs), max(cs) + 1
            nc.vector.tensor_add(ada_pp[:, lo:hi, :], ada_pp[:, lo:hi, :],
                                 bada_pp[:, lo:hi, :])
            for c in cs:
                if c in (1, 4):
                    nc.vector.tensor_scalar_add(ada_pp[:, c, :],
                                                ada_pp[:, c, :], 1.0)
                if c in (0, 1, 3, 4):
                    nc.vector.tensor_scalar_mul(ada_pp[:, c, :],
                                                ada_pp[:, c, :], AS)

        with tc.tile_pool(name="p1w", bufs=1) as p1w, \
             tc.tile_pool(name="pxin", bufs=3) as pxin, \
             tc.tile_pool(name="ps_pro", bufs=2, space="PSUM") as ps_pro, \
             tc.tile_pool(name="ps_tr", bufs=2, space="PSUM") as ps_tr:

            def emit_transpose_block(tt):
                xin = pxin.tile([128, D], F32, tag="xin", name="xin")
                nc.sync.dma_start(
                    xin[:, :], ins["x"][tt * 128:(tt + 1) * 128, :])
                for kd in range(KT):
                    pt = ps_tr.tile([128, 128], F32, tag="ptr", name="pt")
                    nc.tensor.transpose(
                        pt[:, :], xin[:, kd * 128:(kd + 1) * 128], ident[:, :])
                    tsl = slice(tt * 128, (tt + 1) * 128)
                    if kd % 2 == 0:
                        nc.vector.tensor_copy(xT[:, kd, tsl], pt[:, :])
                    else:
                        nc.scalar.copy(xT[:, kd, tsl], pt[:, :])

            for i in range(8):
                emit_transpose_block(i)
                if i == 0:
                    emit_bias_loads()
                if i < 6:
                    emit_ada_chunk(i, p1w, ps_pro)
            emit_ada_pp_load([0, 1])

        # ====== phase B part 1: qkv weight loads + converts (emitted before
        # LN1 so SP streams the loads while ada finishes / LN runs) =========
        es_qk = ExitStack()
        pqk8 = es_qk.enter_context(tc.tile_pool(name="pqk8", bufs=1))
        wq8 = pqk8.tile([128, KTP, D], FP8, name="wq8")
        wk8 = pqk8.tile([128, KTP, D], FP8, name="wk8")
        nc.gpsimd.memset(wq8[:, KT, :], 0.0)
        nc.gpsimd.memset(wk8[:, KT, :], 0.0)

        es_att = ExitStack()
        patt = es_att.enter_context(tc.tile_pool(name="patt", bufs=1, side="right"))
        attn_hs = patt.tile([72, H, NT], FP8, name="attn_hs")
        es_wp = ExitStack()
        pwp8 = es_wp.enter_context(
            tc.tile_pool(name="pwp8", bufs=1, side="right"))
        wp8 = pwp8.tile([72, H, D], FP8, name="wp8")
        es_va = ExitStack()
        pva = es_va.enter_context(tc.tile_pool(name="pva", bufs=1, side="right"))
        v_aug = pva.tile([128, NT // 128, H, 97], FP8, name="v_aug")
        nc.gpsimd.memset(v_aug[:, :, :, HD:97], 0.0)
        nc.gpsimd.memset(v_aug[:, :, :, 96:97], 1.0)

        es_b = ExitStack()
        pwst = es_b.enter_context(tc.tile_pool(name="pwst", bufs=1))
        wv8 = pwst.tile([128, KTP, D], FP8, tag="wv8", bufs=1, name="wv8")
        nc.gpsimd.memset(wv8[:, KT, :], 0.0)
        engs = ["act", "dve", "act", "dve", "act", "dve"]
        for j, (dst8, c0) in enumerate(((wq8, 0), (wk8, D), (wv8, 2 * D))):
            for half in range(2):
                msl = slice(half * 576, (half + 1) * 576)
                wst = pwst.tile([128, KT, 576], F32, tag="wst", bufs=2,
                                name="wst")
                nc.sync.dma_start(
                    wst[:, :, :],
                    ins["w_qkv"][:, c0 + half * 576:c0 + (half + 1) * 576]
                    .rearrange("(k p) m -> p k m", p=128),
                )
                eng = engs[j * 2 + half]
                for kk in range(3):
                    ksl = slice(kk * 3, kk * 3 + 3)
                    if eng == "act":
                        nc.scalar.activation(
                            dst8[:, ksl, msl], wst[:, ksl, :],
                            AF.Identity, scale=WS)
                    elif eng == "dve":
                        nc.vector.tensor_scalar_mul(
                            dst8[:, ksl, msl], wst[:, ksl, :], WS)
                    else:
                        nc.gpsimd.tensor_scalar_mul(
                            dst8[:, ksl, msl], wst[:, ksl, :], WS)

        # ====== LN1 (per-half, interleaved with v matmuls) ==================
        with tc.tile_pool(name="pst", bufs=1) as pst, \
             tc.tile_pool(name="pln", bufs=1) as pln, \
             tc.tile_pool(name="ps_st", bufs=4, space="PSUM") as ps_st, \
             tc.tile_pool(name="ps_v", bufs=3, space="PSUM") as ps_v:

            def v_block(tts):
                for tt in tts:
                    tsl = slice(tt * 128, (tt + 1) * 128)
                    for si, (c0, c1, h0, h1) in enumerate(V_SLICES):
                        pmv = ps_v.tile([128, 512], F32, tag="mv", name="pmv")
                        for i in range(KTP // 2):
                            nc.tensor.matmul(
                                pmv[:, 0:c1 - c0],
                                mod12T[:, 2 * i:2 * i + 2, tsl],
                                wv8[:, 2 * i:2 * i + 2, c0:c1],
                                start=(i == 0), stop=False, perf_mode=DR,
                                skip_group_check=True,
                            )
                        nc.tensor.matmul(
                            pmv[:, 0:c1 - c0], ones_row[:, :],
                            bv_b[:, c0:c1],
                            start=False, stop=True, skip_group_check=True,
                        )
                        vsrc = pmv[:, 0:c1 - c0].rearrange(
                            "p (h d) -> p h d", d=HD)
                        nc.vector.tensor_scalar_mul(
                            v_aug[:, tt, h0:h1, 0:HD], vsrc, 1.0 / (AS * WS))

            st1 = {}
            _ln_stats(tc, nc, xT, ones_col, pst, pln, ps_st, halves=(0,),
                      st=st1)
            _ln_apply(tc, nc, xT, mod12T, st1, ada_pp, 0, 1, pln, halves=(0,))
            _ln_stats(tc, nc, xT, ones_col, pst, pln, ps_st, halves=(1,),
                      st=st1)
            v_block(range(0, 4))
            _ln_apply(tc, nc, xT, mod12T, st1, ada_pp, 0, 1, pln, halves=(1,))
            v_block(range(4, 8))
        es_b.close()

        # ================= phase C: attention ===============================
        with tc.tile_pool(name="p3w", bufs=1) as p3w, \
             tc.tile_pool(name="pexp", bufs=1) as pexp, \
             tc.tile_pool(name="pat3", bufs=1) as pat3, \
             tc.tile_pool(name="ps_qk", bufs=1, space="PSUM") as ps_qk, \
             tc.tile_pool(name="ps_s", bufs=2, space="PSUM") as ps_s, \
             tc.tile_pool(name="ps_av", bufs=2, space="PSUM") as ps_av, \
             tc.tile_pool(name="ps_pa", bufs=1, space="PSUM") as ps_pa:

            def emit_fc1_stream(j):
                f1st = p3w.tile([128, KT, 256], F32, tag="f1st",
                                bufs=2, name="f1st")
                nc.sync.dma_start(
                    f1st[:, :, :],
                    ins["w_fc1"][:, j * 256:(j + 1) * 256]
                    .rearrange("(k p) m -> p k m", p=128),
                )
                f18o = p3w.tile([128, KT, 256], FP8, tag="f18o",
                                bufs=2, name="f18o")
                nc.gpsimd.tensor_scalar_mul(
                    f18o[:, :, :], f1st[:, :, :], WS)
                nc.scalar.dma_start(w1f8_dr[j, :, :, :], f18o[:, :, :])

            def emit_wp_stream(c):
                # reuse the f1st staging tag: [128, KT*256] bytes == 16*144
                wpt = p3w.tile([128, KT, 256], F32, tag="f1st", bufs=2,
                               name="wpt")
                wpv = (wpt[:, :, :].rearrange("p k m -> p (k m)")[0:72, :]
                       .rearrange("p (h m) -> p h m", h=H))
                msl = slice(c * 144, (c + 1) * 144)
                nc.sync.dma_start(
                    wpv[:, :, :],
                    ins["w_proj"][:, msl].rearrange("(h p) m -> p h m", p=72),
                )
                nc.vector.tensor_scalar_mul(
                    wp8[:, :, msl], wpv[:, :, :], WS)

            def emit_wp_stream(c):
                # reuse the f1st staging tag: KT*256 f32 bytes == 16*144
                wpt = p3w.tile([128, KT, 256], F32, tag="f1st", bufs=2,
                               name="wpt")
                wpv = (wpt[:, :, :].rearrange("p k m -> p (k m)")[0:72, :]
                       .rearrange("p (h m) -> p h m", h=H))
                msl = slice(c * 144, (c + 1) * 144)
                nc.sync.dma_start(
                    wpv[:, :, :],
                    ins["w_proj"][:, msl].rearrange("(h p) m -> p h m", p=72),
                )
                nc.vector.tensor_scalar_mul(
                    wp8[:, :, msl], wpv[:, :, :], WS)

            def emit_filler(h):
                # late ada chunks; fc1 fp8 stream-convert to DRAM
                if h % 4 != 3:
                    emit_ada_chunk(6 + h - h // 4, p3w, ps_pa, ps_bufs=1)
                if h == 15:
                    emit_ada_pp_load([2, 3])
                    emit_ada_pp_load([4, 5])
                if 2 <= h:
                    js = ([2 * h - 4, 2 * h - 3] if h < 6
                          else [h + 2])
                    for j in js:
                        emit_fc1_stream(j)
                if h >= 12:
                    emit_wp_stream(h - 12)

            for h in range(H):
                emit_filler(h)
                q_h = pat3.tile([72, NT], FP8, tag="qh", bufs=2, name="q_h")
                k_h = pat3.tile([72, NT], FP8, tag="kh", bufs=2, name="k_h")
                for n in range(2):
                    nsl = slice(n * 512, (n + 1) * 512)
                    pq = ps_qk.tile([72, 512], F32, tag="qk", name="pq")
                    for i in range(KTP // 2):
                        nc.tensor.matmul(
                            pq[:, :],
                            wq8[:, 2 * i:2 * i + 2, h * HD:(h + 1) * HD],
                            mod12T[:, 2 * i:2 * i + 2, nsl],
                            start=(i == 0), stop=(i == KTP // 2 - 1),
                            perf_mode=DR,
                        )
                    nc.vector.tensor_scalar(
                        q_h[:, nsl], pq[:, :], QS / (AS * WS),
                        bq_s[:, h:h + 1], ALU.mult, ALU.add,
                    )
                for n in range(2):
                    nsl = slice(n * 512, (n + 1) * 512)
                    pk = ps_qk.tile([72, 512], F32, tag="qk", name="pk")
                    for i in range(KTP // 2):
                        nc.tensor.matmul(
                            pk[:, :],
                            wk8[:, 2 * i:2 * i + 2, h * HD:(h + 1) * HD],
                            mod12T[:, 2 * i:2 * i + 2, nsl],
                            start=(i == 0), stop=(i == KTP // 2 - 1),
                            perf_mode=DR,
                        )
                    nc.vector.tensor_scalar(
                        k_h[:, nsl], pk[:, :], QS / (AS * WS),
                        bk_s[:, h:h + 1], ALU.mult, ALU.add,
                    )
                for n in range(2):
                    nsl = slice(n * 512, (n + 1) * 512)
                    exp_hn = pexp.tile([128, NT // 128, 512], FP8, tag="exp",
                                       bufs=3, name="exp_hn")
                    for kp in range(NT // 256):
                        pss = ps_s.tile([128, 2, 512], F32, tag="s",
                                        name="pss")
                        for j in range(2):
                            kt_i = 2 * kp + j
                            nc.tensor.matmul(
                                pss[:, j, :],
                                k_h[:, kt_i * 128:(kt_i + 1) * 128],
                                q_h[:, nsl], start=True, stop=True,
                            )
                        nc.scalar.activation(
                            exp_hn[:, 2 * kp:2 * kp + 2, :],
                            pss[:, :, :], AF.Exp, scale=ES)
                    pav = ps_av.tile([97, 512], F32, tag="av", name="pav")
                    for i in range(NT // 256):
                        nc.tensor.matmul(
                            pav[:, :],
                            v_aug[:, 2 * i:2 * i + 2, h, :],
                            exp_hn[:, 2 * i:2 * i + 2, :],
                            start=(i == 0), stop=(i == NT // 256 - 1),
                            perf_mode=DR,
                        )
                    den = pat3.tile([1, 512], F32, tag="den", bufs=3,
                                    name="den")
                    nc.vector.tensor_scalar_mul(den[:, :], pav[96:97, :],
                                                1.0 / PS)
                    nc.vector.reciprocal(den[:, :], den[:, :])
                    denB = pat3.tile([72, 512], F32, tag="denB", bufs=3,
                                     name="denB")
                    nc.gpsimd.partition_broadcast(denB[:, :], den[:, :])
                    nc.vector.tensor_mul(
                        attn_hs[:, h, nsl], pav[0:HD, :], denB[:, :])
        es_qk.close()  # wq8/wk8 no longer needed
        es_va.close()

        # ================= phase D: proj + residual + LN2 ===================
        es_w2 = ExitStack()
        pw2 = es_w2.enter_context(
            tc.tile_pool(name="pw2", bufs=1, side="right"))
        w2f8 = pw2.tile([128, KT, MH, 128], FP8, name="w2f8")

        with tc.tile_pool(name="p4", bufs=1) as p4, \
             tc.tile_pool(name="pst4", bufs=1) as pst4, \
             tc.tile_pool(name="pln4", bufs=1) as pln4:

            for i in range(4, 8):
                msl = slice(i * 144, (i + 1) * 144)
                wpst = p4.tile([72, H, 144], F32, tag="wpst", bufs=2,
                               name="wpst")
                nc.sync.dma_start(
                    wpst[:, :, :],
                    ins["w_proj"][:, msl].rearrange("(h p) m -> p h m", p=72),
                )
                for kk in range(2):
                    hsl = slice(kk * 8, kk * 8 + 8)
                    nc.vector.tensor_scalar_mul(
                        wp8[:, hsl, msl], wpst[:, hsl, :], WS)

            def emit_fc2_chunk(ch, eng, pool):
                f2s = pool.tile([128, MH, 64], F32, tag="f2s", bufs=2,
                                name="f2s")
                nc.sync.dma_start(
                    f2s[:, :, :],
                    ins["w_fc2"][:, ch * 64:(ch + 1) * 64]
                    .rearrange("(k p) m -> p k m", p=128),
                )
                eng.tensor_scalar_mul(
                    w2f8[:, ch // 2, :, (ch % 2) * 64:(ch % 2 + 1) * 64],
                    f2s[:, :, :], WS)

            st2 = {}
            with tc.tile_pool(name="ps_mm2", bufs=4, space="PSUM") as ps_mm2, \
                 tc.tile_pool(name="ps_st2", bufs=4, space="PSUM") as ps_st2:
                for n in range(2):
                    nsl = slice(n * 512, (n + 1) * 512)
                    for mo in range(KT):
                        if mo < 4:
                            ch = n * 4 + mo
                            eng = nc.gpsimd if ch % 2 else nc.vector
                            emit_fc2_chunk(ch, eng, p4)
                        pm2 = ps_mm2.tile([128, 512], F32, tag="mm2",
                                          name="pm2")
                        for i in range(H // 2):
                            nc.tensor.matmul(
                                pm2[:, :],
                                wp8[:, 2 * i:2 * i + 2,
                                    mo * 128:(mo + 1) * 128],
                                attn_hs[:, 2 * i:2 * i + 2, nsl],
                                start=(i == 0), stop=(i == H // 2 - 1),
                                perf_mode=DR,
                            )
                        t_sb = p4.tile([128, 512], F32, tag="tsb", bufs=3,
                                       name="t_sb")
                        nc.scalar.activation(
                            t_sb[:, :], pm2[:, :], AF.Identity,
                            bias=bproj_pp[:, mo:mo + 1], scale=1.0 / (PS * WS),
                        )
                        nc.vector.scalar_tensor_tensor(
                            xT[:, mo, nsl], t_sb[:, :],
                            ada_pp[:, 2, mo:mo + 1], xT[:, mo, nsl],
                            ALU.mult, ALU.add,
                        )
                    _ln_stats(tc, nc, xT, ones_col, pst4, pln4, ps_st2,
                              halves=(n,), st=st2)
                    _ln_apply(tc, nc, xT, mod12T, st2, ada_pp, 3, 4, pln4,
                              halves=(n,))

        # ================= phase E: FFN =====================================
        es_e = ExitStack()
        ph = es_e.enter_context(tc.tile_pool(name="ph", bufs=1))
        hT = ph.tile([128, MH, NT], FP8, name="hT")
        po = es_e.enter_context(tc.tile_pool(name="po", bufs=1))

        with tc.tile_pool(name="ps_f1", bufs=3, space="PSUM") as ps_f1, \
             tc.tile_pool(name="ps_f2", bufs=3, space="PSUM") as ps_f2, \
             tc.tile_pool(name="ps_tro", bufs=2, space="PSUM") as ps_tro:
            # fc1 in 18 chunks of 256 columns (2 m-tiles each), weights
            # already converted to fp8 in DRAM during the attention window
            with tc.tile_pool(name="p5a", bufs=1) as p5a:
                for ch in range(18):
                    f18 = p5a.tile([128, KTP, 256], FP8, tag="f18", bufs=3,
                                   name="f18")
                    nc.sync.dma_start(f18[:, 0:KT, :], w1f8_dr[ch, :, :, :])
                    nc.gpsimd.memset(f18[:, KT, :], 0.0)
                    if 2 <= ch < 12:
                        emit_fc2_chunk(6 + ch,
                                       nc.gpsimd if ch % 2 else nc.vector,
                                       p5a)
                    for m in range(2):
                        mo = ch * 2 + m
                        for n in range(2):
                            nsl = slice(n * 512, (n + 1) * 512)
                            pf1 = ps_f1.tile([128, 512], F32, tag="f1",
                                             name="pf1")
                            for i in range(KTP // 2):
                                nc.tensor.matmul(
                                    pf1[:, :],
                                    f18[:, 2 * i:2 * i + 2,
                                        m * 128:(m + 1) * 128],
                                    mod12T[:, 2 * i:2 * i + 2, nsl],
                                    start=(i == 0), stop=(i == KTP // 2 - 1),
                                    perf_mode=DR,
                                )
                            nc.scalar.activation(
                                hT[:, mo, nsl], pf1[:, :], AF.Gelu_apprx_tanh,
                                bias=bfc1_pp[:, mo:mo + 1],
                                scale=1.0 / (AS * WS),
                            )
            # fc2: weights already fp8-resident in SBUF (w2f8)
            with tc.tile_pool(name="p5b", bufs=1) as p5b:
                for mo in range(KT):
                    for n in range(2):
                        nsl = slice(n * 512, (n + 1) * 512)
                        pf2 = ps_f2.tile([128, 512], F32, tag="f2", name="pf2")
                        for i in range(MH // 2):
                            nc.tensor.matmul(
                                pf2[:, :], w2f8[:, mo, 2 * i:2 * i + 2, :],
                                hT[:, 2 * i:2 * i + 2, nsl],
                                start=(i == 0), stop=(i == MH // 2 - 1),
                                perf_mode=DR,
                            )
                        t2 = p5b.tile([128, 512], F32, tag="t2", bufs=4,
                                      name="t2")
                        nc.scalar.activation(
                            t2[:, :], pf2[:, :], AF.Identity,
                            bias=bfc2_pp[:, mo:mo + 1], scale=1.0 / WS,
                        )
                        nc.vector.scalar_tensor_tensor(
                            xT[:, mo, nsl], t2[:, :], ada_pp[:, 5, mo:mo + 1],
                            xT[:, mo, nsl], ALU.mult, ALU.add,
                        )
                    o_slab = po.tile([128, NT // 128, 128], F32, tag="osl",
                                     bufs=3, name="o_slab")
                    for tt in range(NT // 128):
                        pt = ps_tro.tile([128, 128], F32, tag="tro",
                                         name="pt6")
                        nc.tensor.transpose(
                            pt[:, :], xT[:, mo, tt * 128:(tt + 1) * 128],
                            ident[:, :],
                        )
                        dst = o_slab[:, tt, :]
                        nc.vector.tensor_copy(dst, pt[:, :])
                    nc.scalar.dma_start(
                        out_dram[:, mo * 128:(mo + 1) * 128]
                        .rearrange("(t p) m -> p t m", p=128),
                        o_slab[:, :, :])
        es_w2.close()
        es_wp.close()
        es_att.close()
        es_e.close()


_LOCK = threading.Lock()
_PROG = None


def _get_program():
    global _PROG
    with _LOCK:
        if _PROG is None:
            _PROG = _build_program()
    return _PROG


def _make_in_maps(inputs):
    arrs = {k: np.ascontiguousarray(np.asarray(v, dtype=np.float32))
            for k, v in inputs.items()}
    in_maps = []
    for c in range(NCORES):
        m = {k: v for k, v in arrs.items() if k not in ("x", "t_emb")}
        m["x"] = np.ascontiguousarray(arrs["x"][c])
        m["t_emb"] = np.ascontiguousarray(arrs["t_emb"][c])
        in_maps.append(m)
    return in_maps


def kernel(**inputs):
    nc = _get_program()
    res = run_bass_kernel_spmd(nc, _make_in_maps(inputs),
                               core_ids=list(range(NCORES)))
    return np.stack([r["out"] for r in res.results], axis=0)


def kernel_traced(inputs, **kw):
    """test-harness helper: returns full BassKernelResults with trace."""
    nc = _get_program()
    return run_bass_kernel_spmd(
        nc, _make_in_maps(inputs), core_ids=list(range(NCORES)), trace=True,
        **kw
    )


# revision 121
# speedup vs baseline: 1.6221x; 1.0013x over previous
"""DiT block kernel for Trainium2 (Bass/Tile), 8-core data parallel.

Shapes (hardcoded from the problem spec):
  x: (8, 1024, 1152), t_emb: (8, 1152)
  w_qkv (1152, 3456), w_proj (1152, 1152), w_fc1 (1152, 4608),
  w_fc2 (4608, 1152), w_ada (1152, 6912) + biases.

Strategy: batch-parallel across 8 cores (one batch element each, no
collectives). Activations live transposed [D on partitions, tokens free].
The large matmuls (qkv, attention AV, proj, fc1, fc2) run in fp8e4 with
DoubleRow perf mode (two 128-row k-tiles contracted per instruction);
scale factors for fp8 range are folded into the existing activation
bias/scale stages so no extra elementwise work is added.  LayerNorm
statistics reduce over the partition axis via ones-vector f32r matmuls;
softmax runs transposed (keys on partitions) with denominators collected
through a ones-column appended to V and a fused divide.  q/k are produced
per-head directly (M=72 matmuls cost the same per column as M=128), so
attention needs no partition-crossing gather DMAs.  Weights stream
through big staged f32 DMA loads (few, large transfers) and are
converted on-chip; ada (error-sensitive) stays f32r.
"""

import threading
from contextlib import ExitStack

import numpy as np

import concourse.bass as bass
import concourse.mybir as mybir
import concourse.tile as tile
from concourse import bacc
from concourse.bass_utils import run_bass_kernel_spmd
from concourse.masks import make_identity

F32 = mybir.dt.float32
F32R = mybir.dt.float32r
BF16 = mybir.dt.bfloat16
FP8 = mybir.dt.float8e4
AF = mybir.ActivationFunctionType
ALU = mybir.AluOpType
DR = mybir.MatmulPerfMode.DoubleRow

NCORES = 8
D = 1152
NT = 1024
KT = D // 128       # 9
KTP = KT + 1        # padded to even for DoubleRow pairs
H = 16
HD = 72
HID = 4 * D
MH = HID // 128     # 36
EPS = 1e-6
ISC = 1.0 / float(np.sqrt(HD))

# fp8 scale factors
WS = 64.0           # weights
AS = 8.0            # modulated activations (mod1/mod2)
QS = 2.0            # q/k
PS = 4.0            # attention output
ES = ISC / (QS * QS)  # exp() input scale applied to the scores psum

# v output column slices aligned to head boundaries
V_SLICES = [(0, 432, 0, 6), (432, 864, 6, 12), (864, 1152, 12, 16)]


def _r(ap):
    return ap.bitcast(F32R)


def _build_program():
    nc = bacc.Bacc(
        "TRN2", target_bir_lowering=False, debug=False, enable_asserts=False
    )
    ins = {}
    ins["x"] = nc.dram_tensor("x", [NT, D], F32, kind="ExternalInput").ap()
    ins["t_emb"] = nc.dram_tensor("t_emb", [D], F32, kind="ExternalInput").ap()
    for name, shape in [
        ("w_qkv", [D, 3 * D]), ("b_qkv", [3 * D]),
        ("w_proj", [D, D]), ("b_proj", [D]),
        ("w_fc1", [D, HID]), ("b_fc1", [HID]),
        ("w_fc2", [HID, D]), ("b_fc2", [D]),
        ("w_ada", [D, 6 * D]), ("b_ada", [6 * D]),
    ]:
        ins[name] = nc.dram_tensor(name, shape, F32, kind="ExternalInput").ap()
    out_dram = nc.dram_tensor("out", [NT, D], F32, kind="ExternalOutput").ap()

    with tile.TileContext(nc) as tc:
        _body(tc, ins, out_dram)
    nc.compile()
    return nc


def _ln_stats(tc, nc, src, ones_col, pst, pln, ps_st, halves=(0, 1),
              st=None):
    """Return st[n] = [mean; rstd] rows [1, 2, 512] per 512-token half,
    reducing over the partition (D) axis of src [128, KT, NT] f32."""
    ps_x, ps_q = {}, {}
    if st is None:
        st = {}
    for n in halves:
        nsl = slice(n * 512, (n + 1) * 512)
        ps_x[n] = ps_st.tile([1, 512], F32, tag="st", name=f"psx{n}")
        ps_q[n] = ps_st.tile([1, 512], F32, tag="st", name=f"psq{n}")
        for k in range(KT):
            xb = pln.tile([128, 512], BF16, tag="xb", bufs=2, name="xb")
            nc.scalar.copy(xb[:, :], src[:, k, nsl])
            sq = pln.tile([128, 512], BF16, tag="sq", bufs=2, name="sq")
            nc.vector.tensor_mul(sq[:, :], src[:, k, nsl], src[:, k, nsl])
            nc.tensor.matmul(
                ps_x[n][:, :], ones_col[:, :], xb[:, :],
                start=(k == 0), stop=(k == KT - 1), skip_group_check=True,
            )
            nc.tensor.matmul(
                ps_q[n][:, :], ones_col[:, :], sq[:, :],
                start=(k == 0), stop=(k == KT - 1), skip_group_check=True,
            )
    eps_sb = pst.tile([1, 1], F32, tag="eps", bufs=1, name="eps_sb")
    nc.vector.memset(eps_sb[:, :], EPS)
    for n in halves:
        st[n] = pst.tile([1, 2, 512], F32, tag="lnst", bufs=2, name=f"st{n}")
        nc.vector.tensor_scalar_mul(st[n][:, 0, :], ps_x[n][:, :], 1.0 / D)
        work = pst.tile([1, 512], F32, tag="lnwork", bufs=2, name="work")
        nc.vector.tensor_mul(work[:, :], st[n][:, 0, :], st[n][:, 0, :])
        nc.vector.scalar_tensor_tensor(
            st[n][:, 1, :], ps_q[n][:, :], 1.0 / D, work[:, :],
            ALU.mult, ALU.subtract,
        )
        nc.scalar.activation(st[n][:, 1, :], st[n][:, 1, :], AF.Sqrt,
                             bias=eps_sb[:, :], scale=1.0)
        nc.vector.reciprocal(st[n][:, 1, :], st[n][:, 1, :])
    return st


def _ln_apply(tc, nc, src, dst, st, ada_pp, sh_c, sc_c, pln,
              halves=(0, 1)):
    """dst[:,k,nsl] (fp8) = ((src-mean)*rstd) * ada[sc_c] + ada[sh_c]
    (ada params pre-scaled by AS)."""
    for n in halves:
        nsl = slice(n * 512, (n + 1) * 512)
        meanB = pln.tile([128, 512], F32, tag="meanB", bufs=2, name="meanB")
        rstdB = pln.tile([128, 512], F32, tag="rstdB", bufs=2, name="rstdB")
        nc.gpsimd.partition_broadcast(meanB[:, :], st[n][:, 0, :])
        nc.gpsimd.partition_broadcast(rstdB[:, :], st[n][:, 1, :])
        for k in range(KT):
            t1 = pln.tile([128, 512], F32, tag="lnt1", bufs=3, name="t1")
            nc.vector.tensor_sub(t1[:, :], src[:, k, nsl], meanB[:, :])
            nc.vector.tensor_mul(t1[:, :], t1[:, :], rstdB[:, :])
            nc.gpsimd.tensor_scalar(
                dst[:, k, nsl], t1[:, :],
                ada_pp[:, sc_c, k:k + 1], ada_pp[:, sh_c, k:k + 1],
                ALU.mult, ALU.add,
            )


def _body(tc, ins, out_dram):
    nc = tc.nc
    ctx = ExitStack()
    with ctx:
        dram = ctx.enter_context(tc.tile_pool(name="dram", bufs=1, space="DRAM"))
        ada_dr = dram.tile([6 * D], F32)
        w1f8_dr = dram.tile([18, 128, KT, 256], FP8)

        pers = ctx.enter_context(tc.tile_pool(name="pers", bufs=1))
        ident = pers.tile([128, 128], F32)
        make_identity(nc, ident[:, :])
        ones_col = pers.tile([128, 1], BF16)
        nc.vector.memset(ones_col[:, :], 1.0)
        ones_row = pers.tile([1, 128], BF16)
        nc.vector.memset(ones_row[:, :], 1.0)

        t_pp = pers.tile([128, KT], F32)
        nc.sync.dma_start(t_pp[:, :], ins["t_emb"].rearrange("(k p) -> p k", p=128))
        t_pr = pers.tile([128, KT], F32R)
        nc.scalar.activation(t_pr[:, :], t_pp[:, :], AF.Silu)

        bq_s = pers.tile([72, H], F32)
        bk_s = pers.tile([72, H], F32)
        bv_row = pers.tile([1, D], F32)
        bv_b = pers.tile([1, D], BF16)
        bproj_pp = pers.tile([128, KT], F32)
        bfc1_pp = pers.tile([128, MH], F32)
        bfc2_pp = pers.tile([128, KT], F32)
        bada_pp = pers.tile([128, 6, KT], F32)
        ada_pp = pers.tile([128, 6, KT], F32)

        def emit_bias_loads():
            nc.sync.dma_start(
                bq_s[:, :], ins["b_qkv"][0:D].rearrange("(h p) -> p h", p=72))
            nc.sync.dma_start(
                bk_s[:, :], ins["b_qkv"][D:2 * D].rearrange("(h p) -> p h", p=72))
            nc.sync.dma_start(
                bv_row[:, :],
                ins["b_qkv"][2 * D:3 * D].rearrange("(a b) -> a b", a=1))
            # bv enters the v accumulation in (AS*WS)-scaled psum units
            nc.vector.tensor_scalar_mul(bv_b[:, :], bv_row[:, :], AS * WS)
            nc.sync.dma_start(
                bproj_pp[:, :], ins["b_proj"].rearrange("(m p) -> p m", p=128))
            nc.sync.dma_start(
                bfc1_pp[:, :], ins["b_fc1"].rearrange("(m p) -> p m", p=128))
            nc.sync.dma_start(
                bfc2_pp[:, :], ins["b_fc2"].rearrange("(m p) -> p m", p=128))
            nc.sync.dma_start(
                bada_pp[:, :, :],
                ins["b_ada"].rearrange("(c k p) -> p c k", k=KT, p=128))
            # pre-scale q/k biases by QS (folded into the psum->fp8 copies)
            nc.vector.tensor_scalar_mul(bq_s[:, :], bq_s[:, :], QS)
            nc.vector.tensor_scalar_mul(bk_s[:, :], bk_s[:, :], QS)

        xT = pers.tile([128, KT, NT], F32)      # becomes x2T after residual 1
        mod12T = pers.tile([128, KTP, NT], FP8)  # mod1T, later reused as mod2T
        nc.gpsimd.memset(mod12T[:, KT, :], 0.0)  # DoubleRow pad k-tile

        # ================= phase A: x load/transpose, ada, LN1 ==============

        def emit_ada_chunk(c, p1w, ps_pro, ps_bufs=2):
            """chunk c covers w_ada cols [c*384, (c+1)*384); param p=c//3."""
            wst = p1w.tile([128, KT, 384], F32R, tag="adast", bufs=2, name="wst")
            nc.sync.dma_start(
                wst[:, :, :],
                ins["w_ada"][:, c * 384:(c + 1) * 384]
                .rearrange("(k p) m -> p k m", p=128).bitcast(F32R),
            )
            pa = ps_pro.tile([1, 384], F32, tag="psada", bufs=ps_bufs,
                             name="pa")
            for k in range(KT):
                nc.tensor.matmul(
                    pa[:, :], t_pr[:, k:k + 1], wst[:, k, :],
                    start=(k == 0), stop=(k == KT - 1),
                )
            asb = p1w.tile([1, 384], F32, tag="asb", bufs=3, name="asb")
            nc.vector.tensor_copy(asb[:, :], pa[:, :])
            nc.scalar.dma_start(
                ada_dr[c * 384:(c + 1) * 384].rearrange("(a b) -> a b", a=1),
                asb[0:1, :],
            )

        def emit_ada_pp_load(cs):
            """Load+finalize ada params cs (list) into ada_pp; params 0/1
            (shift_a/scale_a) and 3/4 are pre-scaled by AS; 1/4 get +1."""
            for c in cs:
                nc.scalar.dma_start(
                    ada_pp[:, c, :],
                    ada_dr[c * D:(c + 1) * D].rearrange("(k p) -> p k", p=128),
                )
            lo, hi = min(cs), max(cs) + 1
            nc.vector.tensor_add(ada_pp[:, lo:hi, :], ada_pp[:, lo:hi, :],
                                 bada_pp[:, lo:hi, :])
            for c in cs:
                if c in (1, 4):
                    nc.vector.tensor_scalar_add(ada_pp[:, c, :],
                                                ada_pp[:, c, :], 1.0)
                if c in (0, 1, 3, 4):
                    nc.vector.tensor_scalar_mul(ada_pp[:, c, :],
                                                ada_pp[:, c, :], AS)

        with tc.tile_pool(name="p1w", bufs=1) as p1w, \
             tc.tile_pool(name="pxin", bufs=4) as pxin, \
             tc.tile_pool(name="ps_pro", bufs=2, space="PSUM") as ps_pro, \
             tc.tile_pool(name="ps_tr", bufs=2, space="PSUM") as ps_tr:

            def emit_transpose_block(tt):
                xin = pxin.tile([128, D], F32, tag="xin", name="xin")
                nc.sync.dma_start(
                    xin[:, :], ins["x"][tt * 128:(tt + 1) * 128, :])
                for kd in range(KT):
                    pt = ps_tr.tile([128, 128], F32, tag="ptr", name="pt")
                    nc.tensor.transpose(
                        pt[:, :], xin[:, kd * 128:(kd + 1) * 128], ident[:, :])
                    tsl = slice(tt * 128, (tt + 1) * 128)
                    if kd % 2 == 0:
                        nc.vector.tensor_copy(xT[:, kd, tsl], pt[:, :])
                    else:
                        nc.scalar.copy(xT[:, kd, tsl], pt[:, :])

            for i in range(8):
                emit_transpose_block(i)
                if i == 0:
                    emit_bias_loads()
                if i < 6:
                    emit_ada_chunk(i, p1w, ps_pro)
            emit_ada_pp_load([0, 1])

        # ====== phase B part 1: qkv weight loads + converts (emitted before
        # LN1 so SP streams the loads while ada finishes / LN runs) =========
        es_qk = ExitStack()
        pqk8 = es_qk.enter_context(tc.tile_pool(name="pqk8", bufs=1))
        wq8 = pqk8.tile([128, KTP, D], FP8, name="wq8")
        wk8 = pqk8.tile([128, KTP, D], FP8, name="wk8")
        nc.gpsimd.memset(wq8[:, KT, :], 0.0)
        nc.gpsimd.memset(wk8[:, KT, :], 0.0)

        es_att = ExitStack()
        patt = es_att.enter_context(tc.tile_pool(name="patt", bufs=1, side="right"))
        attn_hs = patt.tile([72, H, NT], FP8, name="attn_hs")
        es_wp = ExitStack()
        pwp8 = es_wp.enter_context(
            tc.tile_pool(name="pwp8", bufs=1, side="right"))
        wp8 = pwp8.tile([72, H, D], FP8, name="wp8")
        es_va = ExitStack()
        pva = es_va.enter_context(tc.tile_pool(name="pva", bufs=1, side="right"))
        v_aug = pva.tile([128, NT // 128, H, 97], FP8, name="v_aug")
        nc.gpsimd.memset(v_aug[:, :, :, HD:97], 0.0)
        nc.gpsimd.memset(v_aug[:, :, :, 96:97], 1.0)

        es_b = ExitStack()
        pwst = es_b.enter_context(tc.tile_pool(name="pwst", bufs=1))
        wv8 = pwst.tile([128, KTP, D], FP8, tag="wv8", bufs=1, name="wv8")
        nc.gpsimd.memset(wv8[:, KT, :], 0.0)
        engs = ["act", "dve", "act", "dve", "act", "dve"]
        for j, (dst8, c0) in enumerate(((wq8, 0), (wk8, D), (wv8, 2 * D))):
            for half in range(2):
                msl = slice(half * 576, (half + 1) * 576)
                wst = pwst.tile([128, KT, 576], F32, tag="wst", bufs=2,
                                name="wst")
                nc.sync.dma_start(
                    wst[:, :, :],
                    ins["w_qkv"][:, c0 + half * 576:c0 + (half + 1) * 576]
                    .rearrange("(k p) m -> p k m", p=128),
                )
                eng = engs[j * 2 + half]
                for kk in range(3):
                    ksl = slice(kk * 3, kk * 3 + 3)
                    if eng == "act":
                        nc.scalar.activation(
                            dst8[:, ksl, msl], wst[:, ksl, :],
                            AF.Identity, scale=WS)
                    elif eng == "dve":
                        nc.vector.tensor_scalar_mul(
                            dst8[:, ksl, msl], wst[:, ksl, :], WS)
                    else:
                        nc.gpsimd.tensor_scalar_mul(
                            dst8[:, ksl, msl], wst[:, ksl, :], WS)

        # ====== LN1 (per-half, interleaved with v matmuls) ==================
        with tc.tile_pool(name="pst", bufs=1) as pst, \
             tc.tile_pool(name="pln", bufs=1) as pln, \
             tc.tile_pool(name="ps_st", bufs=4, space="PSUM") as ps_st, \
             tc.tile_pool(name="ps_v", bufs=4, space="PSUM") as ps_v:

            def v_block(tts):
                for tt in tts:
                    tsl = slice(tt * 128, (tt + 1) * 128)
                    for si, (c0, c1, h0, h1) in enumerate(V_SLICES):
                        pmv = ps_v.tile([128, 512], F32, tag="mv", name="pmv")
                        for i in range(KTP // 2):
                            nc.tensor.matmul(
                                pmv[:, 0:c1 - c0],
                                mod12T[:, 2 * i:2 * i + 2, tsl],
                                wv8[:, 2 * i:2 * i + 2, c0:c1],
                                start=(i == 0), stop=False, perf_mode=DR,
                                skip_group_check=True,
                            )
                        nc.tensor.matmul(
                            pmv[:, 0:c1 - c0], ones_row[:, :],
                            bv_b[:, c0:c1],
                            start=False, stop=True, skip_group_check=True,
                        )
                        vsrc = pmv[:, 0:c1 - c0].rearrange(
                            "p (h d) -> p h d", d=HD)
                        nc.vector.tensor_scalar_mul(
                            v_aug[:, tt, h0:h1, 0:HD], vsrc, 1.0 / (AS * WS))

            st1 = {}
            _ln_stats(tc, nc, xT, ones_col, pst, pln, ps_st, halves=(0,),
                      st=st1)
            _ln_apply(tc, nc, xT, mod12T, st1, ada_pp, 0, 1, pln, halves=(0,))
            _ln_stats(tc, nc, xT, ones_col, pst, pln, ps_st, halves=(1,),
                      st=st1)
            v_block(range(0, 4))
            _ln_apply(tc, nc, xT, mod12T, st1, ada_pp, 0, 1, pln, halves=(1,))
            v_block(range(4, 8))
        es_b.close()

        # ================= phase C: attention ===============================
        with tc.tile_pool(name="p3w", bufs=1) as p3w, \
             tc.tile_pool(name="pexp", bufs=1) as pexp, \
             tc.tile_pool(name="pat3", bufs=1) as pat3, \
             tc.tile_pool(name="ps_qk", bufs=1, space="PSUM") as ps_qk, \
             tc.tile_pool(name="ps_s", bufs=2, space="PSUM") as ps_s, \
             tc.tile_pool(name="ps_av", bufs=2, space="PSUM") as ps_av, \
             tc.tile_pool(name="ps_pa", bufs=1, space="PSUM") as ps_pa:

            def emit_fc1_stream(j):
                f1st = p3w.tile([128, KT, 256], F32, tag="f1st",
                                bufs=2, name="f1st")
                nc.sync.dma_start(
                    f1st[:, :, :],
                    ins["w_fc1"][:, j * 256:(j + 1) * 256]
                    .rearrange("(k p) m -> p k m", p=128),
                )
                f18o = p3w.tile([128, KT, 256], FP8, tag="f18o",
                                bufs=2, name="f18o")
                nc.gpsimd.tensor_scalar_mul(
                    f18o[:, :, :], f1st[:, :, :], WS)
                nc.scalar.dma_start(w1f8_dr[j, :, :, :], f18o[:, :, :])

            def emit_wp_stream(c):
                # reuse the f1st staging tag: [128, KT*256] bytes == 16*144
                wpt = p3w.tile([128, KT, 256], F32, tag="f1st", bufs=2,
                               name="wpt")
                wpv = (wpt[:, :, :].rearrange("p k m -> p (k m)")[0:72, :]
                       .rearrange("p (h m) -> p h m", h=H))
                msl = slice(c * 144, (c + 1) * 144)
                nc.sync.dma_start(
                    wpv[:, :, :],
                    ins["w_proj"][:, msl].rearrange("(h p) m -> p h m", p=72),
                )
                nc.vector.tensor_scalar_mul(
                    wp8[:, :, msl], wpv[:, :, :], WS)

            def emit_wp_stream(c):
                # reuse the f1st staging tag: KT*256 f32 bytes == 16*144
                wpt = p3w.tile([128, KT, 256], F32, tag="f1st", bufs=2,
                               name="wpt")
                wpv = (wpt[:, :, :].rearrange("p k m -> p (k m)")[0:72, :]
                       .rearrange("p (h m) -> p h m", h=H))
                msl = slice(c * 144, (c + 1) * 144)
                nc.sync.dma_start(
                    wpv[:, :, :],
                    ins["w_proj"][:, msl].rearrange("(h p) m -> p h m", p=72),
                )
                nc.vector.tensor_scalar_mul(
                    wp8[:, :, msl], wpv[:, :, :], WS)

            def emit_filler(h):
                # late ada chunks; fc1 fp8 stream-convert to DRAM
                if h % 4 != 3:
                    emit_ada_chunk(6 + h - h // 4, p3w, ps_pa, ps_bufs=1)
                if h == 15:
                    emit_ada_pp_load([2, 3])
                    emit_ada_pp_load([4, 5])
                if 2 <= h:
                    js = ([2 * h - 4, 2 * h - 3] if h < 6
                          else [h + 2])
                    for j in js:
                        emit_fc1_stream(j)
                if h >= 12:
                    emit_wp_stream(h - 12)

            for h in range(H):
                emit_filler(h)
                q_h = pat3.tile([72, NT], FP8, tag="qh", bufs=2, name="q_h")
                k_h = pat3.tile([72, NT], FP8, tag="kh", bufs=2, name="k_h")
                for n in range(2):
                    nsl = slice(n * 512, (n + 1) * 512)
                    pq = ps_qk.tile([72, 512], F32, tag="qk", name="pq")
                    for i in range(KTP // 2):
                        nc.tensor.matmul(
                            pq[:, :],
                            wq8[:, 2 * i:2 * i + 2, h * HD:(h + 1) * HD],
                            mod12T[:, 2 * i:2 * i + 2, nsl],
                            start=(i == 0), stop=(i == KTP // 2 - 1),
                            perf_mode=DR,
                        )
                    nc.vector.tensor_scalar(
                        q_h[:, nsl], pq[:, :], QS / (AS * WS),
                        bq_s[:, h:h + 1], ALU.mult, ALU.add,
                    )
                for n in range(2):
                    nsl = slice(n * 512, (n + 1) * 512)
                    pk = ps_qk.tile([72, 512], F32, tag="qk", name="pk")
                    for i in range(KTP // 2):
                        nc.tensor.matmul(
                            pk[:, :],
                            wk8[:, 2 * i:2 * i + 2, h * HD:(h + 1) * HD],
                            mod12T[:, 2 * i:2 * i + 2, nsl],
                            start=(i == 0), stop=(i == KTP // 2 - 1),
                            perf_mode=DR,
                        )
                    nc.vector.tensor_scalar(
                        k_h[:, nsl], pk[:, :], QS / (AS * WS),
                        bk_s[:, h:h + 1], ALU.mult, ALU.add,
                    )
                for n in range(2):
                    nsl = slice(n * 512, (n + 1) * 512)
                    exp_hn = pexp.tile([128, NT // 128, 512], FP8, tag="exp",
                                       bufs=3, name="exp_hn")
                    for kp in range(NT // 256):
                        pss = ps_s.tile([128, 2, 512], F32, tag="s",
                                        name="pss")
                        for j in range(2):
                            kt_i = 2 * kp + j
                            nc.tensor.matmul(
                                pss[:, j, :],
                                k_h[:, kt_i * 128:(kt_i + 1) * 128],
                                q_h[:, nsl], start=True, stop=True,
                            )
                        nc.scalar.activation(
                            exp_hn[:, 2 * kp:2 * kp + 2, :],
                            pss[:, :, :], AF.Exp, scale=ES)
                    pav = ps_av.tile([97, 512], F32, tag="av", name="pav")
                    for i in range(NT // 256):
                        nc.tensor.matmul(
                            pav[:, :],
                            v_aug[:, 2 * i:2 * i + 2, h, :],
                            exp_hn[:, 2 * i:2 * i + 2, :],
                            start=(i == 0), stop=(i == NT // 256 - 1),
                            perf_mode=DR,
                        )
                    den = pat3.tile([1, 512], F32, tag="den", bufs=3,
                                    name="den")
                    nc.vector.tensor_scalar_mul(den[:, :], pav[96:97, :],
                                                1.0 / PS)
                    nc.vector.reciprocal(den[:, :], den[:, :])
                    denB = pat3.tile([72, 512], F32, tag="denB", bufs=3,
                                     name="denB")
                    nc.gpsimd.partition_broadcast(denB[:, :], den[:, :])
                    nc.vector.tensor_mul(
                        attn_hs[:, h, nsl], pav[0:HD, :], denB[:, :])
        es_qk.close()  # wq8/wk8 no longer needed
        es_va.close()

        # ================= phase D: proj + residual + LN2 ===================
        es_w2 = ExitStack()
        pw2 = es_w2.enter_context(
            tc.tile_pool(name="pw2", bufs=1, side="right"))
        w2f8 = pw2.tile([128, KT, MH, 128], FP8, name="w2f8")

        with tc.tile_pool(name="p4", bufs=1) as p4, \
             tc.tile_pool(name="pst4", bufs=1) as pst4, \
             tc.tile_pool(name="pln4", bufs=1) as pln4:

            for i in range(4, 8):
                msl = slice(i * 144, (i + 1) * 144)
                wpst = p4.tile([72, H, 144], F32, tag="wpst", bufs=2,
                               name="wpst")
                nc.sync.dma_start(
                    wpst[:, :, :],
                    ins["w_proj"][:, msl].rearrange("(h p) m -> p h m", p=72),
                )
                for kk in range(2):
                    hsl = slice(kk * 8, kk * 8 + 8)
                    nc.vector.tensor_scalar_mul(
                        wp8[:, hsl, msl], wpst[:, hsl, :], WS)

            def emit_fc2_chunk(ch, eng, pool):
                f2s = pool.tile([128, MH, 64], F32, tag="f2s", bufs=2,
                                name="f2s")
                nc.sync.dma_start(
                    f2s[:, :, :],
                    ins["w_fc2"][:, ch * 64:(ch + 1) * 64]
                    .rearrange("(k p) m -> p k m", p=128),
                )
                eng.tensor_scalar_mul(
                    w2f8[:, ch // 2, :, (ch % 2) * 64:(ch % 2 + 1) * 64],
                    f2s[:, :, :], WS)

            st2 = {}
            with tc.tile_pool(name="ps_mm2", bufs=4, space="PSUM") as ps_mm2, \
                 tc.tile_pool(name="ps_st2", bufs=4, space="PSUM") as ps_st2:
                for n in range(2):
                    nsl = slice(n * 512, (n + 1) * 512)
                    for mo in range(KT):
                        if mo < 4:
                            ch = n * 4 + mo
                            eng = nc.gpsimd if ch % 2 else nc.vector
                            emit_fc2_chunk(ch, eng, p4)
                        pm2 = ps_mm2.tile([128, 512], F32, tag="mm2",
                                          name="pm2")
                        for i in range(H // 2):
                            nc.tensor.matmul(
                                pm2[:, :],
                                wp8[:, 2 * i:2 * i + 2,
                                    mo * 128:(mo + 1) * 128],
                                attn_hs[:, 2 * i:2 * i + 2, nsl],
                                start=(i == 0), stop=(i == H // 2 - 1),
                                perf_mode=DR,
                            )
                        t_sb = p4.tile([128, 512], F32, tag="tsb", bufs=3,
                                       name="t_sb")
                        nc.scalar.activation(
                            t_sb[:, :], pm2[:, :], AF.Identity,
                            bias=bproj_pp[:, mo:mo + 1], scale=1.0 / (PS * WS),
                        )
                        nc.vector.scalar_tensor_tensor(
                            xT[:, mo, nsl], t_sb[:, :],
                            ada_pp[:, 2, mo:mo + 1], xT[:, mo, nsl],
                            ALU.mult, ALU.add,
                        )
                    _ln_stats(tc, nc, xT, ones_col, pst4, pln4, ps_st2,
                              halves=(n,), st=st2)
                    _ln_apply(tc, nc, xT, mod12T, st2, ada_pp, 3, 4, pln4,
                              halves=(n,))

        # ================= phase E: FFN =====================================
        es_e = ExitStack()
        ph = es_e.enter_context(tc.tile_pool(name="ph", bufs=1))
        hT = ph.tile([128, MH, NT], FP8, name="hT")
        po = es_e.enter_context(tc.tile_pool(name="po", bufs=1))

        with tc.tile_pool(name="ps_f1", bufs=3, space="PSUM") as ps_f1, \
             tc.tile_pool(name="ps_f2", bufs=3, space="PSUM") as ps_f2, \
             tc.tile_pool(name="ps_tro", bufs=2, space="PSUM") as ps_tro:
            # fc1 in 18 chunks of 256 columns (2 m-tiles each), weights
            # already converted to fp8 in DRAM during the attention window
            with tc.tile_pool(name="p5a", bufs=1) as p5a:
                for ch in range(18):
                    f18 = p5a.tile([128, KTP, 256], FP8, tag="f18", bufs=3,
                                   name="f18")
                    nc.sync.dma_start(f18[:, 0:KT, :], w1f8_dr[ch, :, :, :])
                    nc.gpsimd.memset(f18[:, KT, :], 0.0)
                    if 2 <= ch < 12:
                        emit_fc2_chunk(6 + ch,
                                       nc.gpsimd if ch % 2 else nc.vector,
                                       p5a)
                    for m in range(2):
                        mo = ch * 2 + m
                        for n in range(2):
                            nsl = slice(n * 512, (n + 1) * 512)
                            pf1 = ps_f1.tile([128, 512], F32, tag="f1",
                                             name="pf1")
                            for i in range(KTP // 2):
                                nc.tensor.matmul(
                                    pf1[:, :],
                                    f18[:, 2 * i:2 * i + 2,
                                        m * 128:(m + 1) * 128],
                                    mod12T[:, 2 * i:2 * i + 2, nsl],
                                    start=(i == 0), stop=(i == KTP // 2 - 1),
                                    perf_mode=DR,
                                )
                            nc.scalar.activation(
                                hT[:, mo, nsl], pf1[:, :], AF.Gelu_apprx_tanh,
                                bias=bfc1_pp[:, mo:mo + 1],
                                scale=1.0 / (AS * WS),
                            )
            # fc2: weights already fp8-resident in SBUF (w2f8)
            with tc.tile_pool(name="p5b", bufs=1) as p5b:
                for mo in range(KT):
                    for n in range(2):
                        nsl = slice(n * 512, (n + 1) * 512)
                        pf2 = ps_f2.tile([128, 512], F32, tag="f2", name="pf2")
                        for i in range(MH // 2):
                            nc.tensor.matmul(
                                pf2[:, :], w2f8[:, mo, 2 * i:2 * i + 2, :],
                                hT[:, 2 * i:2 * i + 2, nsl],
                                start=(i == 0), stop=(i == MH // 2 - 1),
                                perf_mode=DR,
                            )
                        t2 = p5b.tile([128, 512], F32, tag="t2", bufs=4,
                                      name="t2")
                        nc.scalar.activation(
                            t2[:, :], pf2[:, :], AF.Identity,
                            bias=bfc2_pp[:, mo:mo + 1], scale=1.0 / WS,
                        )
                        nc.vector.scalar_tensor_tensor(
                            xT[:, mo, nsl], t2[:, :], ada_pp[:, 5, mo:mo + 1],
                            xT[:, mo, nsl], ALU.mult, ALU.add,
                        )
                    o_slab = po.tile([128, NT // 128, 128], F32, tag="osl",
                                     bufs=3, name="o_slab")
                    for tt in range(NT // 128):
                        pt = ps_tro.tile([128, 128], F32, tag="tro",
                                         name="pt6")
                        nc.tensor.transpose(
                            pt[:, :], xT[:, mo, tt * 128:(tt + 1) * 128],
                            ident[:, :],
                        )
                        dst = o_slab[:, tt, :]
                        nc.vector.tensor_copy(dst, pt[:, :])
                    nc.scalar.dma_start(
                        out_dram[:, mo * 128:(mo + 1) * 128]
                        .rearrange("(t p) m -> p t m", p=128),
                        o_slab[:, :, :])
        es_w2.close()
        es_wp.close()
        es_att.close()
        es_e.close()


_LOCK = threading.Lock()
_PROG = None


def _get_program():
    global _PROG
    with _LOCK:
        if _PROG is None:
            _PROG = _build_program()
    return _PROG


def _make_in_maps(inputs):
    arrs = {k: np.ascontiguousarray(np.asarray(v, dtype=np.float32))
            for k, v in inputs.items()}
    in_maps = []
    for c in range(NCORES):
        m = {k: v for k, v in arrs.items() if k not in ("x", "t_emb")}
        m["x"] = np.ascontiguousarray(arrs["x"][c])
        m["t_emb"] = np.ascontiguousarray(arrs["t_emb"][c])
        in_maps.append(m)
    return in_maps


def kernel(**inputs):
    nc = _get_program()
    res = run_bass_kernel_spmd(nc, _make_in_maps(inputs),
                               core_ids=list(range(NCORES)))
    return np.stack([r["out"] for r in res.results], axis=0)


def kernel_traced(inputs, **kw):
    """test-harness helper: returns full BassKernelResults with trace."""
    nc = _get_program()
    return run_bass_kernel_spmd(
        nc, _make_in_maps(inputs), core_ids=list(range(NCORES)), trace=True,
        **kw
    )
